# revision 40
# baseline (speedup 1.0000x reference)
"""Distributed Trainium2 kernel for a transformer attention block (B=2, S=4096,
D=1024, H=4096, fp32 I/O).

Reference computation (note the Q<-k, K<-q, V<-v argument quirk):
    k = x @ Wk + bk ; q = x @ Wq + bq ; v = x @ Wv + bv
    scores[s,t] = k[s]·q[t] / sqrt(D); attn = softmax_t(scores) @ v
    x1 = LN(x + attn); h = gelu(x1 @ W1 + b1); out = LN(x1 + h @ W2 + b2)

Sharding: 8 cores -> 2 groups of 4 (one group per batch element); each core
owns 1024 sequence rows. Design notes:
  - all five GEMMs run fp8 (e4m3) DoubleRow matmuls (2x MACs/instruction).
    Weights are pre-cast/pre-tiled on the host into the pair layout DoubleRow
    needs ([p, 2, f] slices with step%16==0). Host pre-scales Wq/Wk/Wv/W1 by
    32 and W2 by 64 so their U(-1/32,1/32)-ish entries leave fp8's subnormal
    range; the inverse scales fold into activation scale constants.
  - gamma/beta of LN1 fold into W1/b1 on the host; the residual stream keeps
    only the normalized z, and gamma/beta(+b2) are re-applied in the FFN2
    epilogue. When gamma==1 and beta==0 (host-detected) the apply passes
    collapse entirely.
  - attention output and FFN2 output are produced in natural [s, d] layout
    (P resp. h are the stationary operand), so both LayerNorms run row-wise
    on the vector engine via bn_stats/bn_aggr -- no PE stat matmuls and no
    output transpose. The softmax reciprocal becomes a per-partition scalar
    after a tiny [8,128] transpose.
  - collectives serialize on one CC stream, so issue order is q-half0,
    q-half1, v; a leading dummy AllGather absorbs the startup barrier skew
    while the x load/transposes run. Biases come pre-packed [128, n] from
    the host (one clean DMA each).
  - softmax rowsum matmuls (DoubleRow, step-16 ones tile) are pipelined one
    chunk behind the score matmuls so the PE never waits on exp.
"""

import sys

if "/opt/trn_rl_repo" not in sys.path:
    sys.path.insert(0, "/opt/trn_rl_repo")

import numpy as np
import ml_dtypes

import concourse.bacc as bacc
import concourse.mybir as mybir
import concourse.tile as tile
from concourse.alu_op_type import AluOpType
from concourse.masks import make_identity


AF = mybir.ActivationFunctionType
FP32 = mybir.dt.float32
BF16 = mybir.dt.bfloat16
FP8 = mybir.dt.float8e4
DR = mybir.MatmulPerfMode.DoubleRow

B, S, D, H = 2, 4096, 1024, 4096
N_CORES = 8
G = 4                 # cores per group (one group per batch element)
S_LOC = S // G        # sequence rows per core
P = 128               # SBUF partitions
NF = 512              # matmul moving free-dim (one fp32 PSUM bank)
DT = D // P           # 8 d-tiles
KP = DT // 2          # 4 k-subtile pairs over D
ST = S_LOC // P       # 8 s-tiles per core
TJ = S // P           # 32 global t-subtiles
HT = H // P           # 32 h-tiles
HG = 4                # FFN1 weight-streaming groups
HPG = HT // HG        # 8 h-tiles per group
EPS = 1e-5
SCL = 32.0            # host pre-scale on Wq/Wk/Wv/W1
SCL2 = 64.0           # host pre-scale on W2
SM_SCALE = 1.0 / float(np.sqrt(np.float32(D)))
EXP_SCALE = SM_SCALE / (SCL * SCL)

GROUPS = [[0, 1, 2, 3], [4, 5, 6, 7]]


def build_graph(nc, tc, ext, trivial_gb):
    stream = ext["stream"]
    persist = ext["persist"]
    stage = ext["stage"]
    const = ext["const"]
    dram = ext["dram"]
    tcx = ext["tc"]

    # ---- constants ----
    ident_bf = const.tile([P, P], BF16, tag="ident_bf", name="ident_bf")
    make_identity(nc, ident_bf[:])
    ident_f = const.tile([P, P], FP32, tag="ident_f", name="ident_f")
    make_identity(nc, ident_f[:])
    ones_dr = const.tile([P, 2, 16], FP8, tag="ones_dr", name="ones_dr")
    nc.vector.memset(ones_dr[:, :, :], 1.0)
    ones_f32 = const.tile([1, P], FP32, tag="ones_f32", name="ones_f32")
    nc.vector.memset(ones_f32[:], 1.0)
    eps_t = const.tile([P, 1], FP32, tag="eps", name="eps")
    nc.vector.memset(eps_t[:], EPS)

    # biases arrive host-packed: [P, 8] bq | [P, 8] bk | [P, 32] b1
    pvecs = const.tile([P, 48], FP32, tag="pvecs", name="pvecs")
    nc.sync.dma_start(out=pvecs[:, 0:DT], in_=ext["bqp_ext"][:, :])
    nc.sync.dma_start(out=pvecs[:, DT:2 * DT], in_=ext["bkp_ext"][:, :])
    nc.sync.dma_start(out=pvecs[:, 2 * DT:2 * DT + HT], in_=ext["b1p_ext"][:, :])
    bq_sb = [pvecs[:, m:m + 1] for m in range(DT)]
    bk_sb = [pvecs[:, DT + m:DT + m + 1] for m in range(DT)]
    b1_sb = [pvecs[:, 2 * DT + m:2 * DT + m + 1] for m in range(HT)]

    # free-dim [1, D] rows at 32-aligned partitions (matmul-legal bases)
    smalls = const.tile([P, D], FP32, tag="smalls", name="smalls")
    SROW = {"bv": 0, "gamma": 32, "beta": 64}
    for nm, r in SROW.items():
        nc.sync.dma_start(out=smalls[r:r + 1, :], in_=ext[nm + "_ext"][0:1, :])
    smalls2 = const.tile([1, D], FP32, tag="smalls2", name="smalls2")
    nc.sync.dma_start(out=smalls2[0:1, :], in_=ext["beta_b2_ext"][0:1, :])

    res = [persist.tile([P, D], FP32, tag=f"res{m}", name=f"res{m}") for m in range(ST)]
    xT_f8 = persist.tile([P, DT, S_LOC], FP8, tag="xT", name="xT")
    qT_f8 = persist.tile([P, DT, S_LOC], FP8, tag="qT", name="qT")
    kT_f8 = persist.tile([P, DT, S_LOC], FP8, tag="kT", name="kT")
    v_half = [persist.tile([P, TJ, NF], FP8, tag=f"vf{h}", name=f"vf{h}")
              for h in range(2)]
    P_f8 = persist.tile([P, TJ, S_LOC], FP8, tag="pf", name="pf")

    ag_q_in = [dram.tile([D, NF], FP8, name=f"agqi{h}") for h in range(2)]
    ag_q_out = [dram.tile([G * D, NF], FP8, name=f"agqo{h}") for h in range(2)]
    ag_v_in = [dram.tile([S_LOC, NF], FP8, name=f"agvi{h}") for h in range(2)]
    ag_v_out = [dram.tile([S, NF], FP8, name=f"agvo{h}") for h in range(2)]

    bcast = {}
    recipT = const.tile([P, ST], FP32, tag="recipT", name="recipT")
    lnt = const.tile([P, 16], FP32, tag="lnt", name="lnt")

    def load_w8(ext_t, base_row):
        tiles = []
        for kp in range(KP):
            wt = stream.tile([P, 2, D], FP8, tag=f"w{kp}", name=f"w{kp}")
            r0 = base_row + kp * P
            nc.sync.dma_start(out=wt[:, :, :], in_=ext_t[r0:r0 + P, :])
            tiles.append(wt)
        return tiles

    def ln_stats(st):
        stats = lnt[:, 0:12]
        nc.vector.bn_stats(stats[:, 0:6], res[st][:, 0:NF])
        nc.vector.bn_stats(stats[:, 6:12], res[st][:, NF:2 * NF])
        mv = lnt[:, 12:14]
        nc.vector.bn_aggr(mv[:], stats[:])
        negmu = lnt[:, 14:15]
        nc.vector.tensor_scalar_mul(negmu[:], mv[:, 0:1], -1.0)
        sd = lnt[:, 15:16]
        nc.scalar.activation(sd[:], mv[:, 1:2], AF.Sqrt, bias=eps_t[:])
        nc.vector.reciprocal(sd[:], sd[:])
        return negmu, sd

    def transpose_to(mmp, src_bf, dst_f8, s0):
        tp = mmp.tile([P, DT * P], BF16, tag="trp", name="trp", bufs=1)
        for dj in range(DT):
            nc.tensor.transpose(
                tp[:, dj * P:(dj + 1) * P], src_bf[:, dj * P:(dj + 1) * P],
                ident_bf[:],
            )
        nc.vector.tensor_copy(
            out=dst_f8[:, :, s0:s0 + P],
            in_=tp[:].rearrange("p (d s) -> p d s", d=DT),
        )

    # ================= phase A: QKV, attention, LN1, FFN1 =================
    with tcx.tile_pool(name="psA", bufs=1, space="PSUM") as mmp:
        # ---- x -> xT fp8: first s-half, then q-half0 can go ----
        def load_x_half(h):
            for si in range(h * 4, h * 4 + 4):
                xn = stage.tile([P, D], FP32, tag="stgf", name="stgf")
                nc.sync.dma_start(out=xn[:], in_=ext["x_ext"][si * P:(si + 1) * P, :])
                xb = stage.tile([P, D], BF16, tag="stgb", name="stgb")
                nc.vector.tensor_copy(out=xb[:], in_=xn[:])
                transpose_to(mmp, xb, xT_f8, si * P)

        def q_half(h):
            n0 = h * NF
            for m in range(DT):
                pt = mmp.tile([P, NF], FP32, tag="mm", name="mm", bufs=4)
                for kp in range(KP):
                    nc.tensor.matmul(
                        pt[:], wq[kp][:, :, m * P:(m + 1) * P],
                        xT_f8[:, 2 * kp:2 * kp + 2, n0:n0 + NF],
                        start=(kp == 0), stop=(kp == KP - 1), perf_mode=DR,
                    )
                nc.scalar.activation(qT_f8[:, m, n0:n0 + NF], pt[:], AF.Identity,
                                     bias=bq_sb[m])
                nc.sync.dma_start(
                    out=ag_q_in[h][m * P:(m + 1) * P, :], in_=qT_f8[:, m, n0:n0 + NF]
                )
            nc.gpsimd.collective_compute(
                "AllGather", AluOpType.bypass, replica_groups=GROUPS,
                ins=[ag_q_in[h][:].opt()], outs=[ag_q_out[h][:].opt()],
            )

        load_x_half(0)
        wq = load_w8(ext["wq8_ext"], 0)
        q_half(0)
        load_x_half(1)
        q_half(1)

        # ---- v = x @ (32 Wv) + 32 bv (natural, fp8); AllGather (CC slot 3) ----
        wv = load_w8(ext["wv8_ext"], 0)
        bv_b = const.tile([P, D], FP32, tag="bc_bv", name="bc_bv")
        for n0 in range(0, D, NF):
            pt = mmp.tile([P, NF], FP32, tag="mm", name="mm", bufs=4)
            nc.tensor.matmul(pt[:], ones_f32[0:1, :], smalls[0:1, n0:n0 + NF])
            nc.scalar.copy(out=bv_b[:, n0:n0 + NF], in_=pt[:])
        for mt in range(ST):
            v8 = stage.tile([P, D], FP8, tag="v8", name="v8")
            for n0 in range(0, D, NF):
                pt = mmp.tile([P, NF], FP32, tag="mm", name="mm", bufs=4)
                for kp in range(KP):
                    nc.tensor.matmul(
                        pt[:], xT_f8[:, 2 * kp:2 * kp + 2, mt * P:(mt + 1) * P],
                        wv[kp][:, :, n0:n0 + NF],
                        start=(kp == 0), stop=(kp == KP - 1), perf_mode=DR,
                    )
                nc.vector.tensor_add(
                    v8[:, n0:n0 + NF], pt[:], bv_b[:, n0:n0 + NF]
                )
            for hh in range(2):
                nc.sync.dma_start(
                    out=ag_v_in[hh][mt * P:(mt + 1) * P, :],
                    in_=v8[:, hh * NF:(hh + 1) * NF],
                )
        # v gathered in two d-halves so pass B's first half can start while
        # the second half is still on the wire (the CC stream is serial)
        for hh in range(2):
            nc.gpsimd.collective_compute(
                "AllGather", AluOpType.bypass, replica_groups=GROUPS,
                ins=[ag_v_in[hh][:].opt()], outs=[ag_v_out[hh][:].opt()],
            )

        # ---- kT = (32 Wk).T @ x + 32 bk (fp8, local) ----
        wk = load_w8(ext["wk8_ext"], 0)
        for m in range(DT):
            for n0 in range(0, S_LOC, NF):
                pt = mmp.tile([P, NF], FP32, tag="mm", name="mm", bufs=4)
                for kp in range(KP):
                    nc.tensor.matmul(
                        pt[:], wk[kp][:, :, m * P:(m + 1) * P],
                        xT_f8[:, 2 * kp:2 * kp + 2, n0:n0 + NF],
                        start=(kp == 0), stop=(kp == KP - 1), perf_mode=DR,
                    )
                nc.scalar.activation(kT_f8[:, m, n0:n0 + NF], pt[:], AF.Identity,
                                     bias=bk_sb[m])

        # [P, D] broadcasts, off the critical path (fills AG wait)
        bc_rows = [("gamma", smalls[32:33, :], ones_f32[0:1, :]),
                   ("beta", smalls[64:65, :], ones_f32[0:1, :]),
                   ("beta_b2", smalls2[0:1, :], ones_f32[0:1, :])]
        if trivial_gb:
            bc_rows = [bc_rows[2]]  # only beta+b2 needed
        for nm, srow, orow in bc_rows:
            bt = const.tile([P, D], FP32, tag=f"bc_{nm}", name=f"bc_{nm}")
            for n0 in range(0, D, NF):
                pt = mmp.tile([P, NF], FP32, tag="mm", name="mm", bufs=4)
                if nm in ("gamma", "beta"):
                    nc.tensor.matmul(pt[:], ones_f32[0:1, :],
                                     smalls[SROW[nm]:SROW[nm] + 1, n0:n0 + NF])
                else:
                    nc.tensor.matmul(pt[:], orow, srow[:, n0:n0 + NF])
                nc.scalar.copy(out=bt[:, n0:n0 + NF], in_=pt[:])
            bcast[nm] = bt

        # ---- pass A: P[t, s] = exp(k·q/sqrt(D)); DR rowsums 1 chunk back ----
        rs_ps = [mmp.tile([1, NF], FP32, tag=f"rs{h}", name=f"rs{h}", bufs=1)
                 for h in range(2)]
        chunks = [(ht, r) for ht in range(2) for r in range(G)]

        def emit_rowsum(ci):
            ht, r = chunks[ci]
            jp0 = (r * ST + ht * 4) // 2
            for h in range(2):
                n0 = h * NF
                for jj in range(2):
                    a = 2 * ci + jj
                    nc.tensor.matmul(
                        rs_ps[h][:], ones_dr[:, :, 0:1],
                        P_f8[:, 2 * (jp0 + jj):2 * (jp0 + jj) + 2, n0:n0 + NF],
                        start=(a == 0), stop=(a == 2 * len(chunks) - 1),
                        perf_mode=DR,
                    )

        qtiles = {}

        def issue_qch(ci):
            ht, r = chunks[ci]
            qch = stream.tile([P, DT, NF], FP8, tag="q", name="q", bufs=3)
            for dsub in range(DT):
                nc.sync.dma_start(
                    out=qch[:, dsub, :],
                    in_=ag_q_out[ht][r * D + dsub * P:r * D + (dsub + 1) * P, :],
                )
            qtiles[ci] = qch

        issue_qch(0)
        for ci, (ht, r) in enumerate(chunks):
            if ci + 1 < len(chunks):
                issue_qch(ci + 1)
            qch = qtiles.pop(ci)
            for tti in range(4):
                j = r * ST + ht * 4 + tti
                for n0 in range(0, S_LOC, NF):
                    ps = mmp.tile([P, NF], FP32, tag="mm", name="mm", bufs=4)
                    for kp in range(KP):
                        nc.tensor.matmul(
                            ps[:], qch[:, 2 * kp:2 * kp + 2, tti * P:(tti + 1) * P],
                            kT_f8[:, 2 * kp:2 * kp + 2, n0:n0 + NF],
                            start=(kp == 0), stop=(kp == KP - 1), perf_mode=DR,
                        )
                    nc.scalar.activation(
                        P_f8[:, j, n0:n0 + NF], ps[:], AF.Exp, scale=EXP_SCALE
                    )
            if ci > 0:
                emit_rowsum(ci - 1)
        emit_rowsum(len(chunks) - 1)

        # recip of rowsums -> rs_row; the tiny transpose to per-partition form
        # is emitted inside pass B (after st0's matmuls) so the PE queue
        # doesn't stall on it before the attention matmuls can start.
        rs_row = const.tile([1, S_LOC], FP32, tag="rs_row", name="rs_row")
        for h in range(2):
            nc.vector.reciprocal(rs_row[0:1, h * NF:(h + 1) * NF], rs_ps[h][:])
        rs8 = const.tile([ST, P], FP32, tag="rs8", name="rs8")
        nc.scalar.dma_start(out=rs8[:, :], in_=rs_row[0:1, :])

        # ---- pass B: attn natural [s, d] + residual -> res (fp32) ----
        # d-half outer: half 0 computes while v's half-1 gather is in flight.
        # The two v loads go to different DMA queues so neither blocks the
        # other or the xre loads behind them.
        nc.scalar.dma_start(
            out=v_half[0][:, :, :],
            in_=ag_v_out[0][:, :].rearrange("(t p) c -> p t c", p=P),
        )
        nc.sync.dma_start(
            out=v_half[1][:, :, :],
            in_=ag_v_out[1][:, :].rearrange("(t p) c -> p t c", p=P),
        )
        for h in range(2):
            n0 = h * NF
            for st in range(ST):
                xre = stage.tile([P, NF], FP32, tag="xre", name="xre")
                nc.scalar.dma_start(
                    out=xre[:], in_=ext["x_ext"][st * P:(st + 1) * P, n0:n0 + NF]
                )
                ps = mmp.tile([P, NF], FP32, tag="mm", name="mm", bufs=4)
                for jp in range(TJ // 2):
                    nc.tensor.matmul(
                        ps[:], P_f8[:, 2 * jp:2 * jp + 2, st * P:(st + 1) * P],
                        v_half[h][:, 2 * jp:2 * jp + 2, :],
                        start=(jp == 0), stop=(jp == TJ // 2 - 1), perf_mode=DR,
                    )
                if h == 0 and st == 0:
                    rt_ps = mmp.tile([P, NF], FP32, tag="mm", name="mm", bufs=4)
                    nc.tensor.transpose(rt_ps[:, 0:ST], rs8[:, :],
                                        ident_f[0:ST, 0:ST])
                    nc.scalar.activation(recipT[:], rt_ps[:, 0:ST], AF.Identity,
                                         scale=1.0 / SCL)
                nc.vector.scalar_tensor_tensor(
                    out=res[st][:, n0:n0 + NF], in0=ps[:], scalar=recipT[:, st:st + 1],
                    in1=xre[:], op0=AluOpType.mult, op1=AluOpType.add,
                )

        # ---- LN1 (stats only -> res = z); x1T fp8; FFN1 per s-half ----
        # h stored per s-half, aliasing the two dead v half-tiles
        x1T_f8 = persist.tile([P, DT, S_LOC], FP8, tag="xT", name="xT")
        h_sh = [persist.tile([P, TJ, NF], FP8, tag=f"vf{h}", name=f"vf{h}")
                for h in range(2)]

        def ln1(st):
            negmu, sd = ln_stats(st)
            nc.vector.tensor_scalar(
                res[st][:], res[st][:], negmu[:], sd[:],
                op0=AluOpType.add, op1=AluOpType.mult,
            )
            xb = stage.tile([P, D], BF16, tag="stgb", name="stgb")
            nc.vector.tensor_copy(out=xb[:], in_=res[st][:])
            transpose_to(mmp, xb, x1T_f8, st * P)

        def ffn1_half(sh):
            n0 = sh * NF
            for g in range(HG):
                w1g = load_w8(ext["w18_ext"], g * KP * P)
                for mh_i in range(HPG):
                    mh = g * HPG + mh_i
                    pt = mmp.tile([P, NF], FP32, tag="mm", name="mm", bufs=4)
                    for kp in range(KP):
                        nc.tensor.matmul(
                            pt[:], w1g[kp][:, :, mh_i * P:(mh_i + 1) * P],
                            x1T_f8[:, 2 * kp:2 * kp + 2, n0:n0 + NF],
                            start=(kp == 0), stop=(kp == KP - 1), perf_mode=DR,
                        )
                    nc.scalar.activation(
                        h_sh[sh][:, mh, :], pt[:], AF.Gelu,
                        bias=b1_sb[mh], scale=1.0 / SCL,
                    )

        for st in range(4):
            ln1(st)
        ffn1_half(0)
        for st in range(4, ST):
            ln1(st)
        ffn1_half(1)

    # ================= phase B: FFN2 (fp8 DR) + LN2 + out =================
    # 4 passes of 2 s-tiles, alternating PSUM bank halves: pass p+1's matmuls
    # overlap pass p's vector epilogues, and only the last pass's tail shows.
    with tcx.tile_pool(name="psB", bufs=1, space="PSUM") as f2p:
        for sp in range(4):
            sts = [2 * sp, 2 * sp + 1]
            bk = 2 * (sp % 2)
            f2 = {(st, h): f2p.tile([P, NF], FP32, tag=f"f{st % 2 + bk}_{h}",
                                    name=f"f{st % 2 + bk}_{h}")
                  for st in sts for h in range(2)}

            def f2mm(kp2, st, h, w2t):
                nc.tensor.matmul(
                    f2[(st, h)][:],
                    h_sh[st // 4][:, 2 * kp2:2 * kp2 + 2,
                                  (st % 4) * P:(st % 4 + 1) * P],
                    w2t[:, :, h * NF:(h + 1) * NF],
                    start=(kp2 == 0), stop=(kp2 == HT // 2 - 1), perf_mode=DR,
                )

            w2_last = None
            for kp2 in range(HT // 2):
                wt = stream.tile([P, 2, D], FP8, tag=f"w{kp2 % KP}",
                                 name=f"w{kp2 % KP}")
                nc.sync.dma_start(
                    out=wt[:, :, :],
                    in_=ext["w28_ext"][kp2 * P:(kp2 + 1) * P, :],
                )
                if kp2 == 0 and trivial_gb:
                    # pre-add beta+b2 into res here (vector is idle during the
                    # matmul stream) so the per-tile tail epilogue shrinks
                    for st in sts:
                        nc.vector.tensor_add(res[st][:], res[st][:],
                                             bcast["beta_b2"][:])
                if kp2 < HT // 2 - 1:
                    for st in sts:
                        for h in range(2):
                            f2mm(kp2, st, h, wt)
                else:
                    w2_last = wt
            # last k-pair: finish one s-tile at a time and stream its epilogue
            for st in sts:
                for h in range(2):
                    f2mm(HT // 2 - 1, st, h, w2_last)
                # pre-LN2 = x1 + ff + b2 = z*gamma + (beta+b2) + f2/SCL2
                if trivial_gb:
                    for h in range(2):
                        n0 = h * NF
                        nc.vector.scalar_tensor_tensor(
                            out=res[st][:, n0:n0 + NF], in0=f2[(st, h)][:],
                            scalar=1.0 / SCL2, in1=res[st][:, n0:n0 + NF],
                            op0=AluOpType.mult, op1=AluOpType.add,
                        )
                else:
                    t2 = stage.tile([P, D], FP32, tag="stgf2", name="stgf2")
                    nc.vector.tensor_mul(t2[:], res[st][:], bcast["gamma"][:])
                    for h in range(2):
                        n0 = h * NF
                        nc.vector.scalar_tensor_tensor(
                            out=t2[:, n0:n0 + NF], in0=f2[(st, h)][:],
                            scalar=1.0 / SCL2, in1=t2[:, n0:n0 + NF],
                            op0=AluOpType.mult, op1=AluOpType.add,
                        )
                    nc.vector.tensor_add(res[st][:], t2[:], bcast["beta_b2"][:])
                # LN2 + store
                negmu, sd = ln_stats(st)
                ot = stage.tile([P, D], FP32, tag="stgf", name="stgf")
                nc.vector.tensor_scalar(
                    ot[:], res[st][:], negmu[:], sd[:],
                    op0=AluOpType.add, op1=AluOpType.mult,
                )
                if not trivial_gb:
                    nc.vector.tensor_mul(ot[:], ot[:], bcast["gamma"][:])
                    nc.vector.tensor_add(ot[:], ot[:], bcast["beta"][:])
                nc.sync.dma_start(
                    out=ext["out_ext"][st * P:(st + 1) * P, :], in_=ot[:]
                )


def build_nc(trivial_gb):
    nc = bacc.Bacc(target_bir_lowering=False, num_devices=N_CORES)

    ext = {
        "x_ext": nc.declare_dram_parameter("x", [S_LOC, D], FP32, isOutput=False),
        "wq8_ext": nc.declare_dram_parameter("wq8", [KP * P, 2 * D], FP8, isOutput=False),
        "wk8_ext": nc.declare_dram_parameter("wk8", [KP * P, 2 * D], FP8, isOutput=False),
        "wv8_ext": nc.declare_dram_parameter("wv8", [KP * P, 2 * D], FP8, isOutput=False),
        "w18_ext": nc.declare_dram_parameter("w18", [HG * KP * P, 2 * D], FP8, isOutput=False),
        "w28_ext": nc.declare_dram_parameter("w28", [(HT // 2) * P, 2 * D], FP8, isOutput=False),
        "bqp_ext": nc.declare_dram_parameter("bqp", [P, DT], FP32, isOutput=False),
        "bkp_ext": nc.declare_dram_parameter("bkp", [P, DT], FP32, isOutput=False),
        "b1p_ext": nc.declare_dram_parameter("b1p", [P, HT], FP32, isOutput=False),
        "bv_ext": nc.declare_dram_parameter("bv", [1, D], FP32, isOutput=False),
        "beta_b2_ext": nc.declare_dram_parameter("beta_b2", [1, D], FP32, isOutput=False),
        "gamma_ext": nc.declare_dram_parameter("gamma", [1, D], FP32, isOutput=False),
        "beta_ext": nc.declare_dram_parameter("beta", [1, D], FP32, isOutput=False),
        "out_ext": nc.declare_dram_parameter("out", [S_LOC, D], FP32, isOutput=True),
    }

    with tile.TileContext(nc) as tc:
        with (
            tc.tile_pool(name="dram", bufs=1, space="DRAM") as dram,
            tc.tile_pool(name="const", bufs=1) as const,
            tc.tile_pool(name="persist", bufs=1) as persist,
            tc.tile_pool(name="stage", bufs=2) as stage,
            tc.tile_pool(name="stream", bufs=2) as stream,
        ):
            ext.update(tc=tc, dram=dram, const=const, persist=persist,
                       stage=stage, stream=stream)
            build_graph(nc, tc, ext, trivial_gb)
    nc.compile()
    return nc


_NC_CACHE = {}


def _get_nc(trivial_gb):
    if trivial_gb not in _NC_CACHE:
        _NC_CACHE[trivial_gb] = build_nc(trivial_gb)
    return _NC_CACHE[trivial_gb]


F8NP = ml_dtypes.float8_e4m3


def _pair_rows(w):
    # [K, N] -> pair layout: rows kp*128+p, cols i*N+c = w[(2kp+i)*128+p, c]
    k, n = w.shape
    kp = k // (2 * P)
    w4 = w.reshape(kp, 2, P, n).transpose(0, 2, 1, 3).reshape(kp * P, 2 * n)
    return np.ascontiguousarray(w4)


def _col_pack(v, n):
    # [n*128] -> [128, n] with out[p, m] = v[m*128 + p]
    return np.ascontiguousarray(v.reshape(n, P).T)


def _make_in_maps(inputs):
    x = np.asarray(inputs["input_embedding"], dtype=np.float32)
    assert x.shape == (B, S, D), x.shape

    gamma = np.asarray(inputs["gamma"], np.float32).reshape(D)
    beta = np.asarray(inputs["beta"], np.float32).reshape(D)
    trivial_gb = bool(np.all(gamma == 1.0) and np.all(beta == 0.0))
    W1 = np.asarray(inputs["W1"], np.float32)
    b1 = np.asarray(inputs["b1"], np.float32).reshape(H)
    # fold LN1's gamma/beta into W1/b1 (FFN1 consumes the normalized z)
    W1f = gamma[:, None] * W1
    b1f = b1 + beta @ W1
    # W1 group-major pair layout: rows (g*KP+kp)*128+p, cols i*D+c
    w1g = (SCL * W1f).reshape(KP, 2, P, HG, D).transpose(3, 0, 2, 1, 4)
    w18 = np.ascontiguousarray(w1g.reshape(HG * KP * P, 2 * D)).astype(F8NP)

    shared = {
        "wq8": _pair_rows(SCL * np.asarray(inputs["Wq"], np.float32)).astype(F8NP),
        "wk8": _pair_rows(SCL * np.asarray(inputs["Wk"], np.float32)).astype(F8NP),
        "wv8": _pair_rows(SCL * np.asarray(inputs["Wv"], np.float32)).astype(F8NP),
        "w18": w18,
        "w28": _pair_rows(SCL2 * np.asarray(inputs["W2"], np.float32)).astype(F8NP),
        "bqp": _col_pack(SCL * np.asarray(inputs["bq"], np.float32).reshape(D), DT),
        "bkp": _col_pack(SCL * np.asarray(inputs["bk"], np.float32).reshape(D), DT),
        "b1p": _col_pack(b1f, HT),
        "bv": SCL * np.asarray(inputs["bv"], np.float32).reshape(1, D),
        "beta_b2": (beta + np.asarray(inputs["b2"], np.float32).reshape(D)).reshape(1, D),
        "gamma": gamma.reshape(1, D),
        "beta": beta.reshape(1, D),
    }

    in_maps = []
    for c in range(N_CORES):
        b = c // G
        r = c % G
        m = dict(shared)
        m["x"] = np.ascontiguousarray(x[b, r * S_LOC:(r + 1) * S_LOC, :])
        in_maps.append(m)
    return in_maps, trivial_gb


def kernel(**inputs: np.ndarray) -> np.ndarray:
    from concourse.bass_utils import run_bass_kernel_spmd

    in_maps, trivial_gb = _make_in_maps(inputs)
    nc = _get_nc(trivial_gb)
    res = run_bass_kernel_spmd(nc, in_maps, core_ids=list(range(N_CORES)))

    out = np.empty((B, S, D), dtype=np.float32)
    for c in range(N_CORES):
        b = c // G
        r = c % G
        out[b, r * S_LOC:(r + 1) * S_LOC, :] = res.results[c]["out"]
    return out


# revision 46
# speedup vs baseline: 1.0335x; 1.0335x over previous
"""Distributed Trainium2 kernel for a transformer attention block (B=2, S=4096,
D=1024, H=4096, fp32 I/O).

Reference computation (note the Q<-k, K<-q, V<-v argument quirk):
    k = x @ Wk + bk ; q = x @ Wq + bq ; v = x @ Wv + bv
    scores[s,t] = k[s]·q[t] / sqrt(D); attn = softmax_t(scores) @ v
    x1 = LN(x + attn); h = gelu(x1 @ W1 + b1); out = LN(x1 + h @ W2 + b2)

Sharding: 8 cores -> 2 groups of 4 (one group per batch element); each core
owns 1024 sequence rows. Design notes:
  - all five GEMMs run fp8 (e4m3) DoubleRow matmuls (2x MACs/instruction).
    Weights are pre-cast/pre-tiled on the host into the pair layout DoubleRow
    needs ([p, 2, f] slices with step%16==0). Host pre-scales Wq/Wk/Wv/W1 by
    32 and W2 by 64 so their U(-1/32,1/32)-ish entries leave fp8's subnormal
    range; the inverse scales fold into activation scale constants.
  - gamma/beta of LN1 fold into W1/b1 on the host; the residual stream keeps
    only the normalized z, and gamma/beta(+b2) are re-applied in the FFN2
    epilogue. When gamma==1 and beta==0 (host-detected) the apply passes
    collapse entirely.
  - attention output and FFN2 output are produced in natural [s, d] layout
    (P resp. h are the stationary operand), so both LayerNorms run row-wise
    on the vector engine via bn_stats/bn_aggr -- no PE stat matmuls and no
    output transpose. The softmax reciprocal becomes a per-partition scalar
    after a tiny [8,128] transpose.
  - collectives serialize on one CC stream, so issue order is q-half0,
    q-half1, v; a leading dummy AllGather absorbs the startup barrier skew
    while the x load/transposes run. Biases come pre-packed [128, n] from
    the host (one clean DMA each).
  - softmax rowsum matmuls (DoubleRow, step-16 ones tile) are pipelined one
    chunk behind the score matmuls so the PE never waits on exp.
"""

import sys

if "/opt/trn_rl_repo" not in sys.path:
    sys.path.insert(0, "/opt/trn_rl_repo")

import numpy as np
import ml_dtypes

import concourse.bacc as bacc
import concourse.mybir as mybir
import concourse.tile as tile
from concourse.alu_op_type import AluOpType
from concourse.masks import make_identity


AF = mybir.ActivationFunctionType
FP32 = mybir.dt.float32
BF16 = mybir.dt.bfloat16
FP8 = mybir.dt.float8e4
DR = mybir.MatmulPerfMode.DoubleRow

B, S, D, H = 2, 4096, 1024, 4096
N_CORES = 8
G = 4                 # cores per group (one group per batch element)
S_LOC = S // G        # sequence rows per core
P = 128               # SBUF partitions
NF = 512              # matmul moving free-dim (one fp32 PSUM bank)
DT = D // P           # 8 d-tiles
KP = DT // 2          # 4 k-subtile pairs over D
ST = S_LOC // P       # 8 s-tiles per core
TJ = S // P           # 32 global t-subtiles
HT = H // P           # 32 h-tiles
HG = 4                # FFN1 weight-streaming groups
HPG = HT // HG        # 8 h-tiles per group
EPS = 1e-5
SCL = 32.0            # host pre-scale on Wq/Wk/Wv/W1
SCL2 = 64.0           # host pre-scale on W2
SM_SCALE = 1.0 / float(np.sqrt(np.float32(D)))
EXP_SCALE = SM_SCALE / (SCL * SCL)

GROUPS = [[0, 1, 2, 3], [4, 5, 6, 7]]


def build_graph(nc, tc, ext, trivial_gb):
    stream = ext["stream"]
    persist = ext["persist"]
    stage = ext["stage"]
    const = ext["const"]
    dram = ext["dram"]
    tcx = ext["tc"]

    # ---- constants ----
    ident_bf = const.tile([P, P], BF16, tag="ident_bf", name="ident_bf")
    make_identity(nc, ident_bf[:])
    ident_f = const.tile([P, P], FP32, tag="ident_f", name="ident_f")
    make_identity(nc, ident_f[:])
    ones_dr = const.tile([P, 2, 16], FP8, tag="ones_dr", name="ones_dr")
    nc.vector.memset(ones_dr[:, :, :], 1.0)
    ones_f32 = const.tile([1, P], FP32, tag="ones_f32", name="ones_f32")
    nc.vector.memset(ones_f32[:], 1.0)
    eps_t = const.tile([P, 1], FP32, tag="eps", name="eps")
    nc.vector.memset(eps_t[:], EPS)

    # biases arrive host-packed: [P, 8] bq | [P, 8] bk | [P, 32] b1
    pvecs = const.tile([P, 48], FP32, tag="pvecs", name="pvecs")
    nc.sync.dma_start(out=pvecs[:, 0:DT], in_=ext["bqp_ext"][:, :])
    nc.sync.dma_start(out=pvecs[:, DT:2 * DT], in_=ext["bkp_ext"][:, :])
    nc.sync.dma_start(out=pvecs[:, 2 * DT:2 * DT + HT], in_=ext["b1p_ext"][:, :])
    bq_sb = [pvecs[:, m:m + 1] for m in range(DT)]
    bk_sb = [pvecs[:, DT + m:DT + m + 1] for m in range(DT)]
    b1_sb = [pvecs[:, 2 * DT + m:2 * DT + m + 1] for m in range(HT)]

    # free-dim [1, D] rows at 32-aligned partitions (matmul-legal bases)
    smalls = const.tile([P, D], FP32, tag="smalls", name="smalls")
    SROW = {"bv": 0, "gamma": 32, "beta": 64}
    for nm, r in SROW.items():
        nc.sync.dma_start(out=smalls[r:r + 1, :], in_=ext[nm + "_ext"][0:1, :])
    smalls2 = const.tile([1, D], FP32, tag="smalls2", name="smalls2")
    nc.sync.dma_start(out=smalls2[0:1, :], in_=ext["beta_b2_ext"][0:1, :])

    res = [persist.tile([P, D], FP32, tag=f"res{m}", name=f"res{m}") for m in range(ST)]
    xT_f8 = persist.tile([P, DT, S_LOC], FP8, tag="xT", name="xT")
    qT_f8 = persist.tile([P, DT, S_LOC], FP8, tag="qT", name="qT")
    kT_f8 = persist.tile([P, DT, S_LOC], FP8, tag="kT", name="kT")
    v_half = [persist.tile([P, TJ, NF], FP8, tag=f"vf{h}", name=f"vf{h}")
              for h in range(2)]
    P_f8 = persist.tile([P, TJ, S_LOC], FP8, tag="pf", name="pf")

    # gather buffers are partition-major ([p, tile, col] flattened) so the
    # post-gather loads are single DMAs with 4KB-contiguous runs
    ag_q_in = [dram.tile([P, DT * NF], FP8, name=f"agqi{h}") for h in range(2)]
    ag_q_out = [dram.tile([G * P, DT * NF], FP8, name=f"agqo{h}") for h in range(2)]
    ag_v_in = [dram.tile([P, ST * NF], FP8, name=f"agvi{h}") for h in range(2)]
    ag_v_out = [dram.tile([G * P, ST * NF], FP8, name=f"agvo{h}") for h in range(2)]

    bcast = {}
    recipT = const.tile([P, ST], FP32, tag="recipT", name="recipT")
    lnt = const.tile([P, 16], FP32, tag="lnt", name="lnt")

    def load_w8(ext_t, base_row):
        tiles = []
        for kp in range(KP):
            wt = stream.tile([P, 2, D], FP8, tag=f"w{kp}", name=f"w{kp}")
            r0 = base_row + kp * P
            nc.sync.dma_start(out=wt[:, :, :], in_=ext_t[r0:r0 + P, :])
            tiles.append(wt)
        return tiles

    def ln_stats(st):
        stats = lnt[:, 0:12]
        nc.vector.bn_stats(stats[:, 0:6], res[st][:, 0:NF])
        nc.vector.bn_stats(stats[:, 6:12], res[st][:, NF:2 * NF])
        mv = lnt[:, 12:14]
        nc.vector.bn_aggr(mv[:], stats[:])
        negmu = lnt[:, 14:15]
        nc.vector.tensor_scalar_mul(negmu[:], mv[:, 0:1], -1.0)
        sd = lnt[:, 15:16]
        nc.scalar.activation(sd[:], mv[:, 1:2], AF.Sqrt, bias=eps_t[:])
        nc.vector.reciprocal(sd[:], sd[:])
        return negmu, sd

    def transpose_to(mmp, src_bf, dst_f8, s0):
        tp = mmp.tile([P, DT * P], BF16, tag="trp", name="trp", bufs=1)
        for dj in range(DT):
            nc.tensor.transpose(
                tp[:, dj * P:(dj + 1) * P], src_bf[:, dj * P:(dj + 1) * P],
                ident_bf[:],
            )
        nc.vector.tensor_copy(
            out=dst_f8[:, :, s0:s0 + P],
            in_=tp[:].rearrange("p (d s) -> p d s", d=DT),
        )

    # ================= phase A: QKV, attention, LN1, FFN1 =================
    with tcx.tile_pool(name="psA", bufs=1, space="PSUM") as mmp:
        # ---- x -> xT fp8: first s-half, then q-half0 can go ----
        def load_x_half(h):
            for si in range(h * 4, h * 4 + 4):
                xn = stage.tile([P, D], FP32, tag="stgf", name="stgf")
                nc.sync.dma_start(out=xn[:], in_=ext["x_ext"][si * P:(si + 1) * P, :])
                xb = stage.tile([P, D], BF16, tag="stgb", name="stgb")
                nc.vector.tensor_copy(out=xb[:], in_=xn[:])
                transpose_to(mmp, xb, xT_f8, si * P)

        def q_half(h):
            n0 = h * NF
            for m in range(DT):
                pt = mmp.tile([P, NF], FP32, tag="mm", name="mm", bufs=4)
                for kp in range(KP):
                    nc.tensor.matmul(
                        pt[:], wq[kp][:, :, m * P:(m + 1) * P],
                        xT_f8[:, 2 * kp:2 * kp + 2, n0:n0 + NF],
                        start=(kp == 0), stop=(kp == KP - 1), perf_mode=DR,
                    )
                nc.scalar.activation(qT_f8[:, m, n0:n0 + NF], pt[:], AF.Identity,
                                     bias=bq_sb[m])
                nc.sync.dma_start(
                    out=ag_q_in[h][:, m * NF:(m + 1) * NF],
                    in_=qT_f8[:, m, n0:n0 + NF],
                )
            nc.gpsimd.collective_compute(
                "AllGather", AluOpType.bypass, replica_groups=GROUPS,
                ins=[ag_q_in[h][:].opt()], outs=[ag_q_out[h][:].opt()],
            )

        load_x_half(0)
        wq = load_w8(ext["wq8_ext"], 0)
        q_half(0)
        load_x_half(1)
        q_half(1)

        # ---- v = x @ (32 Wv) + 32 bv (natural, fp8); AllGather (CC slot 3) ----
        wv = load_w8(ext["wv8_ext"], 0)
        bv_b = const.tile([P, D], FP32, tag="bc_bv", name="bc_bv")
        for n0 in range(0, D, NF):
            pt = mmp.tile([P, NF], FP32, tag="mm", name="mm", bufs=4)
            nc.tensor.matmul(pt[:], ones_f32[0:1, :], smalls[0:1, n0:n0 + NF])
            nc.scalar.copy(out=bv_b[:, n0:n0 + NF], in_=pt[:])
        for mt in range(ST):
            v8 = stage.tile([P, D], FP8, tag="v8", name="v8")
            for n0 in range(0, D, NF):
                pt = mmp.tile([P, NF], FP32, tag="mm", name="mm", bufs=4)
                for kp in range(KP):
                    nc.tensor.matmul(
                        pt[:], xT_f8[:, 2 * kp:2 * kp + 2, mt * P:(mt + 1) * P],
                        wv[kp][:, :, n0:n0 + NF],
                        start=(kp == 0), stop=(kp == KP - 1), perf_mode=DR,
                    )
                nc.vector.tensor_add(
                    v8[:, n0:n0 + NF], pt[:], bv_b[:, n0:n0 + NF]
                )
            for hh in range(2):
                nc.sync.dma_start(
                    out=ag_v_in[hh][:, mt * NF:(mt + 1) * NF],
                    in_=v8[:, hh * NF:(hh + 1) * NF],
                )
        # v gathered in two d-halves so pass B's first half can start while
        # the second half is still on the wire (the CC stream is serial)
        for hh in range(2):
            nc.gpsimd.collective_compute(
                "AllGather", AluOpType.bypass, replica_groups=GROUPS,
                ins=[ag_v_in[hh][:].opt()], outs=[ag_v_out[hh][:].opt()],
            )

        # ---- kT = (32 Wk).T @ x + 32 bk (fp8, local) ----
        wk = load_w8(ext["wk8_ext"], 0)
        for m in range(DT):
            for n0 in range(0, S_LOC, NF):
                pt = mmp.tile([P, NF], FP32, tag="mm", name="mm", bufs=4)
                for kp in range(KP):
                    nc.tensor.matmul(
                        pt[:], wk[kp][:, :, m * P:(m + 1) * P],
                        xT_f8[:, 2 * kp:2 * kp + 2, n0:n0 + NF],
                        start=(kp == 0), stop=(kp == KP - 1), perf_mode=DR,
                    )
                nc.scalar.activation(kT_f8[:, m, n0:n0 + NF], pt[:], AF.Identity,
                                     bias=bk_sb[m])

        # [P, D] broadcasts, off the critical path (fills AG wait)
        bc_rows = [("gamma", smalls[32:33, :], ones_f32[0:1, :]),
                   ("beta", smalls[64:65, :], ones_f32[0:1, :]),
                   ("beta_b2", smalls2[0:1, :], ones_f32[0:1, :])]
        if trivial_gb:
            bc_rows = [bc_rows[2]]  # only beta+b2 needed
        for nm, srow, orow in bc_rows:
            bt = const.tile([P, D], FP32, tag=f"bc_{nm}", name=f"bc_{nm}")
            for n0 in range(0, D, NF):
                pt = mmp.tile([P, NF], FP32, tag="mm", name="mm", bufs=4)
                if nm in ("gamma", "beta"):
                    nc.tensor.matmul(pt[:], ones_f32[0:1, :],
                                     smalls[SROW[nm]:SROW[nm] + 1, n0:n0 + NF])
                else:
                    nc.tensor.matmul(pt[:], orow, srow[:, n0:n0 + NF])
                nc.scalar.copy(out=bt[:, n0:n0 + NF], in_=pt[:])
            bcast[nm] = bt

        # ---- pass A: P[t, s] = exp(k·q/sqrt(D)); DR rowsums 1 chunk back ----
        rs_ps = [mmp.tile([1, NF], FP32, tag=f"rs{h}", name=f"rs{h}", bufs=1)
                 for h in range(2)]
        chunks = [(ht, r) for ht in range(2) for r in range(G)]

        def emit_rowsum(ci):
            ht, r = chunks[ci]
            jp0 = (r * ST + ht * 4) // 2
            for h in range(2):
                n0 = h * NF
                for jj in range(2):
                    a = 2 * ci + jj
                    nc.tensor.matmul(
                        rs_ps[h][:], ones_dr[:, :, 0:1],
                        P_f8[:, 2 * (jp0 + jj):2 * (jp0 + jj) + 2, n0:n0 + NF],
                        start=(a == 0), stop=(a == 2 * len(chunks) - 1),
                        perf_mode=DR,
                    )

        qtiles = {}

        def issue_qch(ci):
            ht, r = chunks[ci]
            qch = stream.tile([P, DT, NF], FP8, tag="q", name="q", bufs=3)
            nc.sync.dma_start(
                out=qch[:, :, :], in_=ag_q_out[ht][r * P:(r + 1) * P, :]
            )
            qtiles[ci] = qch

        issue_qch(0)
        for ci, (ht, r) in enumerate(chunks):
            if ci + 1 < len(chunks):
                issue_qch(ci + 1)
            qch = qtiles.pop(ci)
            for tti in range(4):
                j = r * ST + ht * 4 + tti
                for n0 in range(0, S_LOC, NF):
                    ps = mmp.tile([P, NF], FP32, tag="mm", name="mm", bufs=4)
                    for kp in range(KP):
                        nc.tensor.matmul(
                            ps[:], qch[:, 2 * kp:2 * kp + 2, tti * P:(tti + 1) * P],
                            kT_f8[:, 2 * kp:2 * kp + 2, n0:n0 + NF],
                            start=(kp == 0), stop=(kp == KP - 1), perf_mode=DR,
                        )
                    nc.scalar.activation(
                        P_f8[:, j, n0:n0 + NF], ps[:], AF.Exp, scale=EXP_SCALE
                    )
            if ci > 0:
                emit_rowsum(ci - 1)
        emit_rowsum(len(chunks) - 1)

        # recip of rowsums -> rs_row; the tiny transpose to per-partition form
        # is emitted inside pass B (after st0's matmuls) so the PE queue
        # doesn't stall on it before the attention matmuls can start.
        rs_row = const.tile([1, S_LOC], FP32, tag="rs_row", name="rs_row")
        for h in range(2):
            nc.vector.reciprocal(rs_row[0:1, h * NF:(h + 1) * NF], rs_ps[h][:])
        rs8 = const.tile([ST, P], FP32, tag="rs8", name="rs8")
        nc.scalar.dma_start(out=rs8[:, :], in_=rs_row[0:1, :])

        # ---- pass B: attn natural [s, d] + residual -> res (fp32) ----
        # d-half outer: half 0 computes while v's half-1 gather is in flight.
        # The two v loads go to different DMA queues so neither blocks the
        # other or the xre loads behind them.
        for hh, eng in ((0, nc.sync), (1, nc.scalar)):
            eng.dma_start(
                out=v_half[hh][:, :, :].rearrange("p (r m) c -> p r (m c)", r=G),
                in_=ag_v_out[hh][:, :].rearrange("(r p) c -> p r c", p=P),
            )
        for h in range(2):
            n0 = h * NF
            for st in range(ST):
                xre = stage.tile([P, NF], FP32, tag="xre", name="xre")
                nc.scalar.dma_start(
                    out=xre[:], in_=ext["x_ext"][st * P:(st + 1) * P, n0:n0 + NF]
                )
                ps = mmp.tile([P, NF], FP32, tag="mm", name="mm", bufs=4)
                for jp in range(TJ // 2):
                    nc.tensor.matmul(
                        ps[:], P_f8[:, 2 * jp:2 * jp + 2, st * P:(st + 1) * P],
                        v_half[h][:, 2 * jp:2 * jp + 2, :],
                        start=(jp == 0), stop=(jp == TJ // 2 - 1), perf_mode=DR,
                    )
                if h == 0 and st == 0:
                    rt_ps = mmp.tile([P, NF], FP32, tag="mm", name="mm", bufs=4)
                    nc.tensor.transpose(rt_ps[:, 0:ST], rs8[:, :],
                                        ident_f[0:ST, 0:ST])
                    nc.scalar.activation(recipT[:], rt_ps[:, 0:ST], AF.Identity,
                                         scale=1.0 / SCL)
                nc.vector.scalar_tensor_tensor(
                    out=res[st][:, n0:n0 + NF], in0=ps[:], scalar=recipT[:, st:st + 1],
                    in1=xre[:], op0=AluOpType.mult, op1=AluOpType.add,
                )

        # ---- LN1 (stats only -> res = z); x1T fp8; FFN1 per s-half ----
        # h stored per s-half, aliasing the two dead v half-tiles
        x1T_f8 = persist.tile([P, DT, S_LOC], FP8, tag="xT", name="xT")
        h_sh = [persist.tile([P, TJ, NF], FP8, tag=f"vf{h}", name=f"vf{h}")
                for h in range(2)]

        def ln1(st):
            negmu, sd = ln_stats(st)
            nc.vector.tensor_scalar(
                res[st][:], res[st][:], negmu[:], sd[:],
                op0=AluOpType.add, op1=AluOpType.mult,
            )
            xb = stage.tile([P, D], BF16, tag="stgb", name="stgb")
            nc.vector.tensor_copy(out=xb[:], in_=res[st][:])
            transpose_to(mmp, xb, x1T_f8, st * P)

        def ffn1_half(sh):
            n0 = sh * NF
            for g in range(HG):
                w1g = load_w8(ext["w18_ext"], g * KP * P)
                for mh_i in range(HPG):
                    mh = g * HPG + mh_i
                    pt = mmp.tile([P, NF], FP32, tag="mm", name="mm", bufs=4)
                    for kp in range(KP):
                        nc.tensor.matmul(
                            pt[:], w1g[kp][:, :, mh_i * P:(mh_i + 1) * P],
                            x1T_f8[:, 2 * kp:2 * kp + 2, n0:n0 + NF],
                            start=(kp == 0), stop=(kp == KP - 1), perf_mode=DR,
                        )
                    nc.scalar.activation(
                        h_sh[sh][:, mh, :], pt[:], AF.Gelu,
                        bias=b1_sb[mh], scale=1.0 / SCL,
                    )

        for st in range(4):
            ln1(st)
        ffn1_half(0)
        for st in range(4, ST):
            ln1(st)
        ffn1_half(1)

    # ================= phase B: FFN2 (fp8 DR) + LN2 + out =================
    # 4 passes of 2 s-tiles, alternating PSUM bank halves: pass p+1's matmuls
    # overlap pass p's vector epilogues, and only the last pass's tail shows.
    with tcx.tile_pool(name="psB", bufs=1, space="PSUM") as f2p:
        for sp in range(4):
            sts = [2 * sp, 2 * sp + 1]
            bk = 2 * (sp % 2)
            f2 = {(st, h): f2p.tile([P, NF], FP32, tag=f"f{st % 2 + bk}_{h}",
                                    name=f"f{st % 2 + bk}_{h}")
                  for st in sts for h in range(2)}

            def f2mm(kp2, st, h, w2t):
                nc.tensor.matmul(
                    f2[(st, h)][:],
                    h_sh[st // 4][:, 2 * kp2:2 * kp2 + 2,
                                  (st % 4) * P:(st % 4 + 1) * P],
                    w2t[:, :, h * NF:(h + 1) * NF],
                    start=(kp2 == 0), stop=(kp2 == HT // 2 - 1), perf_mode=DR,
                )

            w2_last = None
            for kp2 in range(HT // 2):
                wt = stream.tile([P, 2, D], FP8, tag=f"w{kp2 % KP}",
                                 name=f"w{kp2 % KP}")
                nc.sync.dma_start(
                    out=wt[:, :, :],
                    in_=ext["w28_ext"][kp2 * P:(kp2 + 1) * P, :],
                )
                if kp2 == 0 and trivial_gb:
                    # pre-add beta+b2 into res here (vector is idle during the
                    # matmul stream) so the per-tile tail epilogue shrinks
                    for st in sts:
                        nc.vector.tensor_add(res[st][:], res[st][:],
                                             bcast["beta_b2"][:])
                if kp2 < HT // 2 - 1:
                    for st in sts:
                        for h in range(2):
                            f2mm(kp2, st, h, wt)
                else:
                    w2_last = wt
            # last k-pair: finish one s-tile at a time and stream its epilogue
            for st in sts:
                for h in range(2):
                    f2mm(HT // 2 - 1, st, h, w2_last)
                # pre-LN2 = x1 + ff + b2 = z*gamma + (beta+b2) + f2/SCL2
                if trivial_gb:
                    for h in range(2):
                        n0 = h * NF
                        nc.vector.scalar_tensor_tensor(
                            out=res[st][:, n0:n0 + NF], in0=f2[(st, h)][:],
                            scalar=1.0 / SCL2, in1=res[st][:, n0:n0 + NF],
                            op0=AluOpType.mult, op1=AluOpType.add,
                        )
                else:
                    t2 = stage.tile([P, D], FP32, tag="stgf2", name="stgf2")
                    nc.vector.tensor_mul(t2[:], res[st][:], bcast["gamma"][:])
                    for h in range(2):
                        n0 = h * NF
                        nc.vector.scalar_tensor_tensor(
                            out=t2[:, n0:n0 + NF], in0=f2[(st, h)][:],
                            scalar=1.0 / SCL2, in1=t2[:, n0:n0 + NF],
                            op0=AluOpType.mult, op1=AluOpType.add,
                        )
                    nc.vector.tensor_add(res[st][:], t2[:], bcast["beta_b2"][:])
                # LN2 + store
                negmu, sd = ln_stats(st)
                ot = stage.tile([P, D], FP32, tag="stgf", name="stgf")
                nc.vector.tensor_scalar(
                    ot[:], res[st][:], negmu[:], sd[:],
                    op0=AluOpType.add, op1=AluOpType.mult,
                )
                if not trivial_gb:
                    nc.vector.tensor_mul(ot[:], ot[:], bcast["gamma"][:])
                    nc.vector.tensor_add(ot[:], ot[:], bcast["beta"][:])
                nc.sync.dma_start(
                    out=ext["out_ext"][st * P:(st + 1) * P, :], in_=ot[:]
                )


def build_nc(trivial_gb):
    nc = bacc.Bacc(target_bir_lowering=False, num_devices=N_CORES)

    ext = {
        "x_ext": nc.declare_dram_parameter("x", [S_LOC, D], FP32, isOutput=False),
        "wq8_ext": nc.declare_dram_parameter("wq8", [KP * P, 2 * D], FP8, isOutput=False),
        "wk8_ext": nc.declare_dram_parameter("wk8", [KP * P, 2 * D], FP8, isOutput=False),
        "wv8_ext": nc.declare_dram_parameter("wv8", [KP * P, 2 * D], FP8, isOutput=False),
        "w18_ext": nc.declare_dram_parameter("w18", [HG * KP * P, 2 * D], FP8, isOutput=False),
        "w28_ext": nc.declare_dram_parameter("w28", [(HT // 2) * P, 2 * D], FP8, isOutput=False),
        "bqp_ext": nc.declare_dram_parameter("bqp", [P, DT], FP32, isOutput=False),
        "bkp_ext": nc.declare_dram_parameter("bkp", [P, DT], FP32, isOutput=False),
        "b1p_ext": nc.declare_dram_parameter("b1p", [P, HT], FP32, isOutput=False),
        "bv_ext": nc.declare_dram_parameter("bv", [1, D], FP32, isOutput=False),
        "beta_b2_ext": nc.declare_dram_parameter("beta_b2", [1, D], FP32, isOutput=False),
        "gamma_ext": nc.declare_dram_parameter("gamma", [1, D], FP32, isOutput=False),
        "beta_ext": nc.declare_dram_parameter("beta", [1, D], FP32, isOutput=False),
        "out_ext": nc.declare_dram_parameter("out", [S_LOC, D], FP32, isOutput=True),
    }

    with tile.TileContext(nc) as tc:
        with (
            tc.tile_pool(name="dram", bufs=1, space="DRAM") as dram,
            tc.tile_pool(name="const", bufs=1) as const,
            tc.tile_pool(name="persist", bufs=1) as persist,
            tc.tile_pool(name="stage", bufs=2) as stage,
            tc.tile_pool(name="stream", bufs=2) as stream,
        ):
            ext.update(tc=tc, dram=dram, const=const, persist=persist,
                       stage=stage, stream=stream)
            build_graph(nc, tc, ext, trivial_gb)
    nc.compile()
    return nc


_NC_CACHE = {}


def _get_nc(trivial_gb):
    if trivial_gb not in _NC_CACHE:
        _NC_CACHE[trivial_gb] = build_nc(trivial_gb)
    return _NC_CACHE[trivial_gb]


F8NP = ml_dtypes.float8_e4m3


def _pair_rows(w):
    # [K, N] -> pair layout: rows kp*128+p, cols i*N+c = w[(2kp+i)*128+p, c]
    k, n = w.shape
    kp = k // (2 * P)
    w4 = w.reshape(kp, 2, P, n).transpose(0, 2, 1, 3).reshape(kp * P, 2 * n)
    return np.ascontiguousarray(w4)


def _col_pack(v, n):
    # [n*128] -> [128, n] with out[p, m] = v[m*128 + p]
    return np.ascontiguousarray(v.reshape(n, P).T)


def _make_in_maps(inputs):
    x = np.asarray(inputs["input_embedding"], dtype=np.float32)
    assert x.shape == (B, S, D), x.shape

    gamma = np.asarray(inputs["gamma"], np.float32).reshape(D)
    beta = np.asarray(inputs["beta"], np.float32).reshape(D)
    trivial_gb = bool(np.all(gamma == 1.0) and np.all(beta == 0.0))
    W1 = np.asarray(inputs["W1"], np.float32)
    b1 = np.asarray(inputs["b1"], np.float32).reshape(H)
    # fold LN1's gamma/beta into W1/b1 (FFN1 consumes the normalized z)
    W1f = gamma[:, None] * W1
    b1f = b1 + beta @ W1
    # W1 group-major pair layout: rows (g*KP+kp)*128+p, cols i*D+c
    w1g = (SCL * W1f).reshape(KP, 2, P, HG, D).transpose(3, 0, 2, 1, 4)
    w18 = np.ascontiguousarray(w1g.reshape(HG * KP * P, 2 * D)).astype(F8NP)

    shared = {
        "wq8": _pair_rows(SCL * np.asarray(inputs["Wq"], np.float32)).astype(F8NP),
        "wk8": _pair_rows(SCL * np.asarray(inputs["Wk"], np.float32)).astype(F8NP),
        "wv8": _pair_rows(SCL * np.asarray(inputs["Wv"], np.float32)).astype(F8NP),
        "w18": w18,
        "w28": _pair_rows(SCL2 * np.asarray(inputs["W2"], np.float32)).astype(F8NP),
        "bqp": _col_pack(SCL * np.asarray(inputs["bq"], np.float32).reshape(D), DT),
        "bkp": _col_pack(SCL * np.asarray(inputs["bk"], np.float32).reshape(D), DT),
        "b1p": _col_pack(b1f, HT),
        "bv": SCL * np.asarray(inputs["bv"], np.float32).reshape(1, D),
        "beta_b2": (beta + np.asarray(inputs["b2"], np.float32).reshape(D)).reshape(1, D),
        "gamma": gamma.reshape(1, D),
        "beta": beta.reshape(1, D),
    }

    in_maps = []
    for c in range(N_CORES):
        b = c // G
        r = c % G
        m = dict(shared)
        m["x"] = np.ascontiguousarray(x[b, r * S_LOC:(r + 1) * S_LOC, :])
        in_maps.append(m)
    return in_maps, trivial_gb


def kernel(**inputs: np.ndarray) -> np.ndarray:
    from concourse.bass_utils import run_bass_kernel_spmd

    in_maps, trivial_gb = _make_in_maps(inputs)
    nc = _get_nc(trivial_gb)
    res = run_bass_kernel_spmd(nc, in_maps, core_ids=list(range(N_CORES)))

    out = np.empty((B, S, D), dtype=np.float32)
    for c in range(N_CORES):
        b = c // G
        r = c % G
        out[b, r * S_LOC:(r + 1) * S_LOC, :] = res.results[c]["out"]
    return out


# revision 47
# speedup vs baseline: 1.0367x; 1.0031x over previous
"""Distributed Trainium2 kernel for a transformer attention block (B=2, S=4096,
D=1024, H=4096, fp32 I/O).

Reference computation (note the Q<-k, K<-q, V<-v argument quirk):
    k = x @ Wk + bk ; q = x @ Wq + bq ; v = x @ Wv + bv
    scores[s,t] = k[s]·q[t] / sqrt(D); attn = softmax_t(scores) @ v
    x1 = LN(x + attn); h = gelu(x1 @ W1 + b1); out = LN(x1 + h @ W2 + b2)

Sharding: 8 cores -> 2 groups of 4 (one group per batch element); each core
owns 1024 sequence rows. Design notes:
  - all five GEMMs run fp8 (e4m3) DoubleRow matmuls (2x MACs/instruction).
    Weights are pre-cast/pre-tiled on the host into the pair layout DoubleRow
    needs ([p, 2, f] slices with step%16==0). Host pre-scales Wq/Wk/Wv/W1 by
    32 and W2 by 64 so their U(-1/32,1/32)-ish entries leave fp8's subnormal
    range; the inverse scales fold into activation scale constants.
  - gamma/beta of LN1 fold into W1/b1 on the host; the residual stream keeps
    only the normalized z, and gamma/beta(+b2) are re-applied in the FFN2
    epilogue. When gamma==1 and beta==0 (host-detected) the apply passes
    collapse entirely.
  - attention output and FFN2 output are produced in natural [s, d] layout
    (P resp. h are the stationary operand), so both LayerNorms run row-wise
    on the vector engine via bn_stats/bn_aggr -- no PE stat matmuls and no
    output transpose. The softmax reciprocal becomes a per-partition scalar
    after a tiny [8,128] transpose.
  - collectives serialize on one CC stream, so issue order is q-half0,
    q-half1, v; a leading dummy AllGather absorbs the startup barrier skew
    while the x load/transposes run. Biases come pre-packed [128, n] from
    the host (one clean DMA each).
  - softmax rowsum matmuls (DoubleRow, step-16 ones tile) are pipelined one
    chunk behind the score matmuls so the PE never waits on exp.
"""

import sys

if "/opt/trn_rl_repo" not in sys.path:
    sys.path.insert(0, "/opt/trn_rl_repo")

import numpy as np
import ml_dtypes

import concourse.bacc as bacc
import concourse.mybir as mybir
import concourse.tile as tile
from concourse.alu_op_type import AluOpType
from concourse.masks import make_identity


AF = mybir.ActivationFunctionType
FP32 = mybir.dt.float32
BF16 = mybir.dt.bfloat16
FP8 = mybir.dt.float8e4
DR = mybir.MatmulPerfMode.DoubleRow

B, S, D, H = 2, 4096, 1024, 4096
N_CORES = 8
G = 4                 # cores per group (one group per batch element)
S_LOC = S // G        # sequence rows per core
P = 128               # SBUF partitions
NF = 512              # matmul moving free-dim (one fp32 PSUM bank)
DT = D // P           # 8 d-tiles
KP = DT // 2          # 4 k-subtile pairs over D
ST = S_LOC // P       # 8 s-tiles per core
TJ = S // P           # 32 global t-subtiles
HT = H // P           # 32 h-tiles
HG = 4                # FFN1 weight-streaming groups
HPG = HT // HG        # 8 h-tiles per group
EPS = 1e-5
SCL = 32.0            # host pre-scale on Wq/Wk/Wv/W1
SCL2 = 64.0           # host pre-scale on W2
SM_SCALE = 1.0 / float(np.sqrt(np.float32(D)))
EXP_SCALE = SM_SCALE / (SCL * SCL)

GROUPS = [[0, 1, 2, 3], [4, 5, 6, 7]]


def build_graph(nc, tc, ext, trivial_gb):
    stream = ext["stream"]
    persist = ext["persist"]
    stage = ext["stage"]
    const = ext["const"]
    dram = ext["dram"]
    tcx = ext["tc"]

    # ---- constants ----
    ident_bf = const.tile([P, P], BF16, tag="ident_bf", name="ident_bf")
    make_identity(nc, ident_bf[:])
    ident_f = const.tile([P, P], FP32, tag="ident_f", name="ident_f")
    make_identity(nc, ident_f[:])
    ones_dr = const.tile([P, 2, 16], FP8, tag="ones_dr", name="ones_dr")
    nc.vector.memset(ones_dr[:, :, :], 1.0)
    ones_f32 = const.tile([1, P], FP32, tag="ones_f32", name="ones_f32")
    nc.vector.memset(ones_f32[:], 1.0)
    eps_t = const.tile([P, 1], FP32, tag="eps", name="eps")
    nc.vector.memset(eps_t[:], EPS)

    # biases arrive host-packed: [P, 8] bq | [P, 8] bk | [P, 32] b1
    pvecs = const.tile([P, 48], FP32, tag="pvecs", name="pvecs")
    nc.sync.dma_start(out=pvecs[:, 0:DT], in_=ext["bqp_ext"][:, :])
    nc.sync.dma_start(out=pvecs[:, DT:2 * DT], in_=ext["bkp_ext"][:, :])
    nc.sync.dma_start(out=pvecs[:, 2 * DT:2 * DT + HT], in_=ext["b1p_ext"][:, :])
    bq_sb = [pvecs[:, m:m + 1] for m in range(DT)]
    bk_sb = [pvecs[:, DT + m:DT + m + 1] for m in range(DT)]
    b1_sb = [pvecs[:, 2 * DT + m:2 * DT + m + 1] for m in range(HT)]

    # free-dim [1, D] rows at 32-aligned partitions (matmul-legal bases)
    smalls = const.tile([P, D], FP32, tag="smalls", name="smalls")
    SROW = {"bv": 0, "gamma": 32, "beta": 64}
    for nm, r in SROW.items():
        nc.sync.dma_start(out=smalls[r:r + 1, :], in_=ext[nm + "_ext"][0:1, :])
    smalls2 = const.tile([1, D], FP32, tag="smalls2", name="smalls2")
    nc.sync.dma_start(out=smalls2[0:1, :], in_=ext["beta_b2_ext"][0:1, :])

    res = [persist.tile([P, D], FP32, tag=f"res{m}", name=f"res{m}") for m in range(ST)]
    xT_f8 = persist.tile([P, DT, S_LOC], FP8, tag="xT", name="xT")
    qT_f8 = persist.tile([P, DT, S_LOC], FP8, tag="qT", name="qT")
    kT_f8 = persist.tile([P, DT, S_LOC], FP8, tag="kT", name="kT")
    v_half = [persist.tile([P, TJ, NF], FP8, tag=f"vf{h}", name=f"vf{h}")
              for h in range(2)]
    P_f8 = persist.tile([P, TJ, S_LOC], FP8, tag="pf", name="pf")

    # gather buffers are partition-major ([p, tile, col] flattened) so the
    # post-gather loads are single DMAs with 4KB-contiguous runs
    ag_q_in = [dram.tile([P, DT * NF], FP8, name=f"agqi{h}") for h in range(2)]
    ag_q_out = [dram.tile([G * P, DT * NF], FP8, name=f"agqo{h}") for h in range(2)]
    ag_v_in = [dram.tile([P, ST * NF], FP8, name=f"agvi{h}") for h in range(2)]
    ag_v_out = [dram.tile([G * P, ST * NF], FP8, name=f"agvo{h}") for h in range(2)]

    bcast = {}
    recipT = const.tile([P, ST], FP32, tag="recipT", name="recipT")
    lnt = const.tile([P, 16], FP32, tag="lnt", name="lnt")

    def load_w8(ext_t, base_row):
        tiles = []
        for kp in range(KP):
            wt = stream.tile([P, 2, D], FP8, tag=f"w{kp}", name=f"w{kp}")
            r0 = base_row + kp * P
            nc.sync.dma_start(out=wt[:, :, :], in_=ext_t[r0:r0 + P, :])
            tiles.append(wt)
        return tiles

    def ln_stats(st):
        stats = lnt[:, 0:12]
        nc.vector.bn_stats(stats[:, 0:6], res[st][:, 0:NF])
        nc.vector.bn_stats(stats[:, 6:12], res[st][:, NF:2 * NF])
        mv = lnt[:, 12:14]
        nc.vector.bn_aggr(mv[:], stats[:])
        negmu = lnt[:, 14:15]
        nc.vector.tensor_scalar_mul(negmu[:], mv[:, 0:1], -1.0)
        sd = lnt[:, 15:16]
        nc.scalar.activation(sd[:], mv[:, 1:2], AF.Sqrt, bias=eps_t[:])
        nc.vector.reciprocal(sd[:], sd[:])
        return negmu, sd

    def transpose_to(mmp, src_bf, dst_f8, s0):
        tp = mmp.tile([P, DT * P], BF16, tag="trp", name="trp", bufs=1)
        for dj in range(DT):
            nc.tensor.transpose(
                tp[:, dj * P:(dj + 1) * P], src_bf[:, dj * P:(dj + 1) * P],
                ident_bf[:],
            )
        nc.vector.tensor_copy(
            out=dst_f8[:, :, s0:s0 + P],
            in_=tp[:].rearrange("p (d s) -> p d s", d=DT),
        )

    # ================= phase A: QKV, attention, LN1, FFN1 =================
    with tcx.tile_pool(name="psA", bufs=1, space="PSUM") as mmp:
        # ---- x -> xT fp8: first s-half, then q-half0 can go ----
        def load_x_half(h):
            for si in range(h * 4, h * 4 + 4):
                xn = stage.tile([P, D], FP32, tag="stgf", name="stgf")
                nc.sync.dma_start(out=xn[:], in_=ext["x_ext"][si * P:(si + 1) * P, :])
                xb = stage.tile([P, D], BF16, tag="stgb", name="stgb")
                nc.vector.tensor_copy(out=xb[:], in_=xn[:])
                transpose_to(mmp, xb, xT_f8, si * P)

        def q_half(h):
            n0 = h * NF
            for m in range(DT):
                pt = mmp.tile([P, NF], FP32, tag="mm", name="mm", bufs=4)
                for kp in range(KP):
                    nc.tensor.matmul(
                        pt[:], wq[kp][:, :, m * P:(m + 1) * P],
                        xT_f8[:, 2 * kp:2 * kp + 2, n0:n0 + NF],
                        start=(kp == 0), stop=(kp == KP - 1), perf_mode=DR,
                    )
                nc.scalar.activation(qT_f8[:, m, n0:n0 + NF], pt[:], AF.Identity,
                                     bias=bq_sb[m])
                nc.sync.dma_start(
                    out=ag_q_in[h][:, m * NF:(m + 1) * NF],
                    in_=qT_f8[:, m, n0:n0 + NF],
                )
            nc.gpsimd.collective_compute(
                "AllGather", AluOpType.bypass, replica_groups=GROUPS,
                ins=[ag_q_in[h][:].opt()], outs=[ag_q_out[h][:].opt()],
            )

        load_x_half(0)
        wq = load_w8(ext["wq8_ext"], 0)
        q_half(0)
        load_x_half(1)
        q_half(1)

        # ---- v = x @ (32 Wv) + 32 bv (natural, fp8); AllGather (CC slot 3) ----
        wv = load_w8(ext["wv8_ext"], 0)
        bv_b = const.tile([P, D], FP32, tag="bc_bv", name="bc_bv")
        for n0 in range(0, D, NF):
            pt = mmp.tile([P, NF], FP32, tag="mm", name="mm", bufs=4)
            nc.tensor.matmul(pt[:], ones_f32[0:1, :], smalls[0:1, n0:n0 + NF])
            nc.scalar.copy(out=bv_b[:, n0:n0 + NF], in_=pt[:])
        for mt in range(ST):
            v8 = stage.tile([P, D], FP8, tag="v8", name="v8")
            for n0 in range(0, D, NF):
                pt = mmp.tile([P, NF], FP32, tag="mm", name="mm", bufs=4)
                for kp in range(KP):
                    nc.tensor.matmul(
                        pt[:], xT_f8[:, 2 * kp:2 * kp + 2, mt * P:(mt + 1) * P],
                        wv[kp][:, :, n0:n0 + NF],
                        start=(kp == 0), stop=(kp == KP - 1), perf_mode=DR,
                    )
                nc.vector.tensor_add(
                    v8[:, n0:n0 + NF], pt[:], bv_b[:, n0:n0 + NF]
                )
            for hh in range(2):
                nc.sync.dma_start(
                    out=ag_v_in[hh][:, mt * NF:(mt + 1) * NF],
                    in_=v8[:, hh * NF:(hh + 1) * NF],
                )
        # v gathered in two d-halves so pass B's first half can start while
        # the second half is still on the wire (the CC stream is serial)
        for hh in range(2):
            nc.gpsimd.collective_compute(
                "AllGather", AluOpType.bypass, replica_groups=GROUPS,
                ins=[ag_v_in[hh][:].opt()], outs=[ag_v_out[hh][:].opt()],
            )

        # ---- kT = (32 Wk).T @ x + 32 bk (fp8, local) ----
        wk = load_w8(ext["wk8_ext"], 0)
        for m in range(DT):
            for n0 in range(0, S_LOC, NF):
                pt = mmp.tile([P, NF], FP32, tag="mm", name="mm", bufs=4)
                for kp in range(KP):
                    nc.tensor.matmul(
                        pt[:], wk[kp][:, :, m * P:(m + 1) * P],
                        xT_f8[:, 2 * kp:2 * kp + 2, n0:n0 + NF],
                        start=(kp == 0), stop=(kp == KP - 1), perf_mode=DR,
                    )
                nc.scalar.activation(kT_f8[:, m, n0:n0 + NF], pt[:], AF.Identity,
                                     bias=bk_sb[m])

        # [P, D] broadcasts, off the critical path (fills AG wait)
        bc_rows = [("gamma", smalls[32:33, :], ones_f32[0:1, :]),
                   ("beta", smalls[64:65, :], ones_f32[0:1, :]),
                   ("beta_b2", smalls2[0:1, :], ones_f32[0:1, :])]
        if trivial_gb:
            bc_rows = [bc_rows[2]]  # only beta+b2 needed
        for nm, srow, orow in bc_rows:
            bt = const.tile([P, D], FP32, tag=f"bc_{nm}", name=f"bc_{nm}")
            for n0 in range(0, D, NF):
                pt = mmp.tile([P, NF], FP32, tag="mm", name="mm", bufs=4)
                if nm in ("gamma", "beta"):
                    nc.tensor.matmul(pt[:], ones_f32[0:1, :],
                                     smalls[SROW[nm]:SROW[nm] + 1, n0:n0 + NF])
                else:
                    nc.tensor.matmul(pt[:], orow, srow[:, n0:n0 + NF])
                nc.scalar.copy(out=bt[:, n0:n0 + NF], in_=pt[:])
            bcast[nm] = bt

        # ---- pass A: P[t, s] = exp(k·q/sqrt(D)); DR rowsums 1 chunk back ----
        rs_ps = [mmp.tile([1, NF], FP32, tag=f"rs{h}", name=f"rs{h}", bufs=1)
                 for h in range(2)]
        chunks = [(ht, r) for ht in range(2) for r in range(G)]

        def emit_rowsum(ci):
            ht, r = chunks[ci]
            jp0 = (r * ST + ht * 4) // 2
            for h in range(2):
                n0 = h * NF
                for jj in range(2):
                    a = 2 * ci + jj
                    nc.tensor.matmul(
                        rs_ps[h][:], ones_dr[:, :, 0:1],
                        P_f8[:, 2 * (jp0 + jj):2 * (jp0 + jj) + 2, n0:n0 + NF],
                        start=(a == 0), stop=(a == 2 * len(chunks) - 1),
                        perf_mode=DR,
                    )

        qtiles = {}

        def issue_qch(ci):
            ht, r = chunks[ci]
            qch = stream.tile([P, DT, NF], FP8, tag="q", name="q", bufs=3)
            nc.sync.dma_start(
                out=qch[:, :, :], in_=ag_q_out[ht][r * P:(r + 1) * P, :]
            )
            qtiles[ci] = qch

        issue_qch(0)
        for ci, (ht, r) in enumerate(chunks):
            if ci + 1 < len(chunks):
                issue_qch(ci + 1)
            qch = qtiles.pop(ci)
            for tti in range(4):
                j = r * ST + ht * 4 + tti
                for n0 in range(0, S_LOC, NF):
                    ps = mmp.tile([P, NF], FP32, tag="mm", name="mm", bufs=4)
                    for kp in range(KP):
                        nc.tensor.matmul(
                            ps[:], qch[:, 2 * kp:2 * kp + 2, tti * P:(tti + 1) * P],
                            kT_f8[:, 2 * kp:2 * kp + 2, n0:n0 + NF],
                            start=(kp == 0), stop=(kp == KP - 1), perf_mode=DR,
                        )
                    nc.scalar.activation(
                        P_f8[:, j, n0:n0 + NF], ps[:], AF.Exp, scale=EXP_SCALE
                    )
            if ci > 0:
                emit_rowsum(ci - 1)
        emit_rowsum(len(chunks) - 1)

        # recip of rowsums -> rs_row; the tiny transpose to per-partition form
        # is emitted inside pass B (after st0's matmuls) so the PE queue
        # doesn't stall on it before the attention matmuls can start.
        rs_row = const.tile([1, S_LOC], FP32, tag="rs_row", name="rs_row")
        for h in range(2):
            nc.vector.reciprocal(rs_row[0:1, h * NF:(h + 1) * NF], rs_ps[h][:])
        rs8 = const.tile([ST, P], FP32, tag="rs8", name="rs8")
        nc.scalar.dma_start(out=rs8[:, :], in_=rs_row[0:1, :])

        # ---- pass B: attn natural [s, d] + residual -> res (fp32) ----
        # d-half outer: half 0 computes while v's half-1 gather is in flight.
        # The two v loads go to different DMA queues so neither blocks the
        # other or the xre loads behind them.
        for hh, eng in ((0, nc.sync), (1, nc.scalar)):
            eng.dma_start(
                out=v_half[hh][:, :, :].rearrange("p (r m) c -> p r (m c)", r=G),
                in_=ag_v_out[hh][:, :].rearrange("(r p) c -> p r c", p=P),
            )
        for h in range(2):
            n0 = h * NF
            for st in range(ST):
                xre = stage.tile([P, NF], FP32, tag="xre", name="xre")
                nc.scalar.dma_start(
                    out=xre[:], in_=ext["x_ext"][st * P:(st + 1) * P, n0:n0 + NF]
                )
                ps = mmp.tile([P, NF], FP32, tag="mm", name="mm", bufs=4)
                for jp in range(TJ // 2):
                    nc.tensor.matmul(
                        ps[:], P_f8[:, 2 * jp:2 * jp + 2, st * P:(st + 1) * P],
                        v_half[h][:, 2 * jp:2 * jp + 2, :],
                        start=(jp == 0), stop=(jp == TJ // 2 - 1), perf_mode=DR,
                    )
                if h == 0 and st == 0:
                    rt_ps = mmp.tile([P, NF], FP32, tag="mm", name="mm", bufs=4)
                    nc.tensor.transpose(rt_ps[:, 0:ST], rs8[:, :],
                                        ident_f[0:ST, 0:ST])
                    nc.scalar.activation(recipT[:], rt_ps[:, 0:ST], AF.Identity,
                                         scale=1.0 / SCL)
                nc.vector.scalar_tensor_tensor(
                    out=res[st][:, n0:n0 + NF], in0=ps[:], scalar=recipT[:, st:st + 1],
                    in1=xre[:], op0=AluOpType.mult, op1=AluOpType.add,
                )

        # ---- LN1 (stats only -> res = z); x1T fp8; FFN1 per s-half ----
        # h stored per s-half, aliasing the two dead v half-tiles
        x1T_f8 = persist.tile([P, DT, S_LOC], FP8, tag="xT", name="xT")
        h_sh = [persist.tile([P, TJ, NF], FP8, tag=f"vf{h}", name=f"vf{h}")
                for h in range(2)]

        def ln1(st):
            negmu, sd = ln_stats(st)
            nc.vector.tensor_scalar(
                res[st][:], res[st][:], negmu[:], sd[:],
                op0=AluOpType.add, op1=AluOpType.mult,
            )
            xb = stage.tile([P, D], BF16, tag="stgb", name="stgb")
            nc.vector.tensor_copy(out=xb[:], in_=res[st][:])
            transpose_to(mmp, xb, x1T_f8, st * P)

        def ffn1_half(sh):
            n0 = sh * NF
            for g in range(HG):
                w1g = load_w8(ext["w18_ext"], g * KP * P)
                for mh_i in range(HPG):
                    mh = g * HPG + mh_i
                    pt = mmp.tile([P, NF], FP32, tag="mm", name="mm", bufs=4)
                    for kp in range(KP):
                        nc.tensor.matmul(
                            pt[:], w1g[kp][:, :, mh_i * P:(mh_i + 1) * P],
                            x1T_f8[:, 2 * kp:2 * kp + 2, n0:n0 + NF],
                            start=(kp == 0), stop=(kp == KP - 1), perf_mode=DR,
                        )
                    nc.scalar.activation(
                        h_sh[sh][:, mh, :], pt[:], AF.Gelu,
                        bias=b1_sb[mh], scale=1.0 / SCL,
                    )

        for st in range(4):
            ln1(st)
        ffn1_half(0)
        for st in range(4, ST):
            ln1(st)
        ffn1_half(1)

    # ================= phase B: FFN2 (fp8 DR) + LN2 + out =================
    # 4 passes of 2 s-tiles, alternating PSUM bank halves: pass p+1's matmuls
    # overlap pass p's vector epilogues, and only the last pass's tail shows.
    with tcx.tile_pool(name="psB", bufs=1, space="PSUM") as f2p:
        passes = [(0, 1), (2, 3), (4, 5), (6,), (7,)]
        for sp, sts in enumerate(passes):
            bk = 2 * (sp % 2)
            f2 = {(st, h): f2p.tile([P, NF], FP32, tag=f"f{i + bk}_{h}",
                                    name=f"f{i + bk}_{h}")
                  for i, st in enumerate(sts) for h in range(2)}

            def f2mm(kp2, st, h, w2t):
                nc.tensor.matmul(
                    f2[(st, h)][:],
                    h_sh[st // 4][:, 2 * kp2:2 * kp2 + 2,
                                  (st % 4) * P:(st % 4 + 1) * P],
                    w2t[:, :, h * NF:(h + 1) * NF],
                    start=(kp2 == 0), stop=(kp2 == HT // 2 - 1), perf_mode=DR,
                )

            w2_last = None
            for kp2 in range(HT // 2):
                wt = stream.tile([P, 2, D], FP8, tag=f"w{kp2 % KP}",
                                 name=f"w{kp2 % KP}")
                nc.sync.dma_start(
                    out=wt[:, :, :],
                    in_=ext["w28_ext"][kp2 * P:(kp2 + 1) * P, :],
                )
                if kp2 == 0 and trivial_gb:
                    # pre-add beta+b2 into res here (vector is idle during the
                    # matmul stream) so the per-tile tail epilogue shrinks
                    for st in sts:
                        nc.vector.tensor_add(res[st][:], res[st][:],
                                             bcast["beta_b2"][:])
                if kp2 < HT // 2 - 1:
                    for st in sts:
                        for h in range(2):
                            f2mm(kp2, st, h, wt)
                else:
                    w2_last = wt
            # last k-pair: finish one s-tile at a time and stream its epilogue
            for st in sts:
                for h in range(2):
                    f2mm(HT // 2 - 1, st, h, w2_last)
                # pre-LN2 = x1 + ff + b2 = z*gamma + (beta+b2) + f2/SCL2
                if trivial_gb:
                    for h in range(2):
                        n0 = h * NF
                        nc.vector.scalar_tensor_tensor(
                            out=res[st][:, n0:n0 + NF], in0=f2[(st, h)][:],
                            scalar=1.0 / SCL2, in1=res[st][:, n0:n0 + NF],
                            op0=AluOpType.mult, op1=AluOpType.add,
                        )
                else:
                    t2 = stage.tile([P, D], FP32, tag="stgf2", name="stgf2")
                    nc.vector.tensor_mul(t2[:], res[st][:], bcast["gamma"][:])
                    for h in range(2):
                        n0 = h * NF
                        nc.vector.scalar_tensor_tensor(
                            out=t2[:, n0:n0 + NF], in0=f2[(st, h)][:],
                            scalar=1.0 / SCL2, in1=t2[:, n0:n0 + NF],
                            op0=AluOpType.mult, op1=AluOpType.add,
                        )
                    nc.vector.tensor_add(res[st][:], t2[:], bcast["beta_b2"][:])
                # LN2 + store
                negmu, sd = ln_stats(st)
                ot = stage.tile([P, D], FP32, tag="stgf", name="stgf")
                nc.vector.tensor_scalar(
                    ot[:], res[st][:], negmu[:], sd[:],
                    op0=AluOpType.add, op1=AluOpType.mult,
                )
                if not trivial_gb:
                    nc.vector.tensor_mul(ot[:], ot[:], bcast["gamma"][:])
                    nc.vector.tensor_add(ot[:], ot[:], bcast["beta"][:])
                nc.sync.dma_start(
                    out=ext["out_ext"][st * P:(st + 1) * P, :], in_=ot[:]
                )


def build_nc(trivial_gb):
    nc = bacc.Bacc(target_bir_lowering=False, num_devices=N_CORES)

    ext = {
        "x_ext": nc.declare_dram_parameter("x", [S_LOC, D], FP32, isOutput=False),
        "wq8_ext": nc.declare_dram_parameter("wq8", [KP * P, 2 * D], FP8, isOutput=False),
        "wk8_ext": nc.declare_dram_parameter("wk8", [KP * P, 2 * D], FP8, isOutput=False),
        "wv8_ext": nc.declare_dram_parameter("wv8", [KP * P, 2 * D], FP8, isOutput=False),
        "w18_ext": nc.declare_dram_parameter("w18", [HG * KP * P, 2 * D], FP8, isOutput=False),
        "w28_ext": nc.declare_dram_parameter("w28", [(HT // 2) * P, 2 * D], FP8, isOutput=False),
        "bqp_ext": nc.declare_dram_parameter("bqp", [P, DT], FP32, isOutput=False),
        "bkp_ext": nc.declare_dram_parameter("bkp", [P, DT], FP32, isOutput=False),
        "b1p_ext": nc.declare_dram_parameter("b1p", [P, HT], FP32, isOutput=False),
        "bv_ext": nc.declare_dram_parameter("bv", [1, D], FP32, isOutput=False),
        "beta_b2_ext": nc.declare_dram_parameter("beta_b2", [1, D], FP32, isOutput=False),
        "gamma_ext": nc.declare_dram_parameter("gamma", [1, D], FP32, isOutput=False),
        "beta_ext": nc.declare_dram_parameter("beta", [1, D], FP32, isOutput=False),
        "out_ext": nc.declare_dram_parameter("out", [S_LOC, D], FP32, isOutput=True),
    }

    with tile.TileContext(nc) as tc:
        with (
            tc.tile_pool(name="dram", bufs=1, space="DRAM") as dram,
            tc.tile_pool(name="const", bufs=1) as const,
            tc.tile_pool(name="persist", bufs=1) as persist,
            tc.tile_pool(name="stage", bufs=2) as stage,
            tc.tile_pool(name="stream", bufs=2) as stream,
        ):
            ext.update(tc=tc, dram=dram, const=const, persist=persist,
                       stage=stage, stream=stream)
            build_graph(nc, tc, ext, trivial_gb)
    nc.compile()
    return nc


_NC_CACHE = {}


def _get_nc(trivial_gb):
    if trivial_gb not in _NC_CACHE:
        _NC_CACHE[trivial_gb] = build_nc(trivial_gb)
    return _NC_CACHE[trivial_gb]


F8NP = ml_dtypes.float8_e4m3


def _pair_rows(w):
    # [K, N] -> pair layout: rows kp*128+p, cols i*N+c = w[(2kp+i)*128+p, c]
    k, n = w.shape
    kp = k // (2 * P)
    w4 = w.reshape(kp, 2, P, n).transpose(0, 2, 1, 3).reshape(kp * P, 2 * n)
    return np.ascontiguousarray(w4)


def _col_pack(v, n):
    # [n*128] -> [128, n] with out[p, m] = v[m*128 + p]
    return np.ascontiguousarray(v.reshape(n, P).T)


def _make_in_maps(inputs):
    x = np.asarray(inputs["input_embedding"], dtype=np.float32)
    assert x.shape == (B, S, D), x.shape

    gamma = np.asarray(inputs["gamma"], np.float32).reshape(D)
    beta = np.asarray(inputs["beta"], np.float32).reshape(D)
    trivial_gb = bool(np.all(gamma == 1.0) and np.all(beta == 0.0))
    W1 = np.asarray(inputs["W1"], np.float32)
    b1 = np.asarray(inputs["b1"], np.float32).reshape(H)
    # fold LN1's gamma/beta into W1/b1 (FFN1 consumes the normalized z)
    W1f = gamma[:, None] * W1
    b1f = b1 + beta @ W1
    # W1 group-major pair layout: rows (g*KP+kp)*128+p, cols i*D+c
    w1g = (SCL * W1f).reshape(KP, 2, P, HG, D).transpose(3, 0, 2, 1, 4)
    w18 = np.ascontiguousarray(w1g.reshape(HG * KP * P, 2 * D)).astype(F8NP)

    shared = {
        "wq8": _pair_rows(SCL * np.asarray(inputs["Wq"], np.float32)).astype(F8NP),
        "wk8": _pair_rows(SCL * np.asarray(inputs["Wk"], np.float32)).astype(F8NP),
        "wv8": _pair_rows(SCL * np.asarray(inputs["Wv"], np.float32)).astype(F8NP),
        "w18": w18,
        "w28": _pair_rows(SCL2 * np.asarray(inputs["W2"], np.float32)).astype(F8NP),
        "bqp": _col_pack(SCL * np.asarray(inputs["bq"], np.float32).reshape(D), DT),
        "bkp": _col_pack(SCL * np.asarray(inputs["bk"], np.float32).reshape(D), DT),
        "b1p": _col_pack(b1f, HT),
        "bv": SCL * np.asarray(inputs["bv"], np.float32).reshape(1, D),
        "beta_b2": (beta + np.asarray(inputs["b2"], np.float32).reshape(D)).reshape(1, D),
        "gamma": gamma.reshape(1, D),
        "beta": beta.reshape(1, D),
    }

    in_maps = []
    for c in range(N_CORES):
        b = c // G
        r = c % G
        m = dict(shared)
        m["x"] = np.ascontiguousarray(x[b, r * S_LOC:(r + 1) * S_LOC, :])
        in_maps.append(m)
    return in_maps, trivial_gb


def kernel(**inputs: np.ndarray) -> np.ndarray:
    from concourse.bass_utils import run_bass_kernel_spmd

    in_maps, trivial_gb = _make_in_maps(inputs)
    nc = _get_nc(trivial_gb)
    res = run_bass_kernel_spmd(nc, in_maps, core_ids=list(range(N_CORES)))

    out = np.empty((B, S, D), dtype=np.float32)
    for c in range(N_CORES):
        b = c // G
        r = c % G
        out[b, r * S_LOC:(r + 1) * S_LOC, :] = res.results[c]["out"]
    return out


# revision 50
# speedup vs baseline: 1.0430x; 1.0062x over previous
"""Distributed Trainium2 kernel for a transformer attention block (B=2, S=4096,
D=1024, H=4096, fp32 I/O).

Reference computation (note the Q<-k, K<-q, V<-v argument quirk):
    k = x @ Wk + bk ; q = x @ Wq + bq ; v = x @ Wv + bv
    scores[s,t] = k[s]·q[t] / sqrt(D); attn = softmax_t(scores) @ v
    x1 = LN(x + attn); h = gelu(x1 @ W1 + b1); out = LN(x1 + h @ W2 + b2)

Sharding: 8 cores -> 2 groups of 4 (one group per batch element); each core
owns 1024 sequence rows. Design notes:
  - all five GEMMs run fp8 (e4m3) DoubleRow matmuls (2x MACs/instruction).
    Weights are pre-cast/pre-tiled on the host into the pair layout DoubleRow
    needs ([p, 2, f] slices with step%16==0). Host pre-scales Wq/Wk/Wv/W1 by
    32 and W2 by 64 so their U(-1/32,1/32)-ish entries leave fp8's subnormal
    range; the inverse scales fold into activation scale constants.
  - gamma/beta of LN1 fold into W1/b1 on the host; the residual stream keeps
    only the normalized z, and gamma/beta(+b2) are re-applied in the FFN2
    epilogue. When gamma==1 and beta==0 (host-detected) the apply passes
    collapse entirely.
  - attention output and FFN2 output are produced in natural [s, d] layout
    (P resp. h are the stationary operand), so both LayerNorms run row-wise
    on the vector engine via bn_stats/bn_aggr -- no PE stat matmuls and no
    output transpose. The softmax reciprocal becomes a per-partition scalar
    after a tiny [8,128] transpose.
  - collectives serialize on one CC stream, so issue order is q-half0,
    q-half1, v; a leading dummy AllGather absorbs the startup barrier skew
    while the x load/transposes run. Biases come pre-packed [128, n] from
    the host (one clean DMA each).
  - softmax rowsum matmuls (DoubleRow, step-16 ones tile) are pipelined one
    chunk behind the score matmuls so the PE never waits on exp.
"""

import sys

if "/opt/trn_rl_repo" not in sys.path:
    sys.path.insert(0, "/opt/trn_rl_repo")

import numpy as np
import ml_dtypes

import concourse.bacc as bacc
import concourse.mybir as mybir
import concourse.tile as tile
from concourse.alu_op_type import AluOpType
from concourse.masks import make_identity


AF = mybir.ActivationFunctionType
FP32 = mybir.dt.float32
BF16 = mybir.dt.bfloat16
FP8 = mybir.dt.float8e4
DR = mybir.MatmulPerfMode.DoubleRow

B, S, D, H = 2, 4096, 1024, 4096
N_CORES = 8
G = 4                 # cores per group (one group per batch element)
S_LOC = S // G        # sequence rows per core
P = 128               # SBUF partitions
NF = 512              # matmul moving free-dim (one fp32 PSUM bank)
DT = D // P           # 8 d-tiles
KP = DT // 2          # 4 k-subtile pairs over D
ST = S_LOC // P       # 8 s-tiles per core
TJ = S // P           # 32 global t-subtiles
HT = H // P           # 32 h-tiles
HG = 4                # FFN1 weight-streaming groups
HPG = HT // HG        # 8 h-tiles per group
EPS = 1e-5
SCL = 32.0            # host pre-scale on Wq/Wk/Wv/W1
SCL2 = 64.0           # host pre-scale on W2
SM_SCALE = 1.0 / float(np.sqrt(np.float32(D)))
EXP_SCALE = SM_SCALE / (SCL * SCL)

GROUPS = [[0, 1, 2, 3], [4, 5, 6, 7]]


def build_graph(nc, tc, ext, trivial_gb):
    stream = ext["stream"]
    persist = ext["persist"]
    stage = ext["stage"]
    const = ext["const"]
    dram = ext["dram"]
    tcx = ext["tc"]

    # ---- constants ----
    ident_bf = const.tile([P, P], BF16, tag="ident_bf", name="ident_bf")
    make_identity(nc, ident_bf[:])
    ident_f = const.tile([P, P], FP32, tag="ident_f", name="ident_f")
    make_identity(nc, ident_f[:])
    ones_dr = const.tile([P, 2, 16], FP8, tag="ones_dr", name="ones_dr")
    nc.vector.memset(ones_dr[:, :, :], 1.0)
    ones_f32 = const.tile([1, P], FP32, tag="ones_f32", name="ones_f32")
    nc.vector.memset(ones_f32[:], 1.0)
    eps_t = const.tile([P, 1], FP32, tag="eps", name="eps")
    nc.vector.memset(eps_t[:], EPS)

    # biases arrive host-packed: [P, 8] bq | [P, 8] bk | [P, 32] b1
    pvecs = const.tile([P, 48], FP32, tag="pvecs", name="pvecs")
    nc.sync.dma_start(out=pvecs[:, 0:DT], in_=ext["bqp_ext"][:, :])
    nc.sync.dma_start(out=pvecs[:, DT:2 * DT], in_=ext["bkp_ext"][:, :])
    nc.sync.dma_start(out=pvecs[:, 2 * DT:2 * DT + HT], in_=ext["b1p_ext"][:, :])
    bq_sb = [pvecs[:, m:m + 1] for m in range(DT)]
    bk_sb = [pvecs[:, DT + m:DT + m + 1] for m in range(DT)]
    b1_sb = [pvecs[:, 2 * DT + m:2 * DT + m + 1] for m in range(HT)]

    # free-dim [1, D] rows at 32-aligned partitions (matmul-legal bases)
    smalls = const.tile([P, D], FP32, tag="smalls", name="smalls")
    SROW = {"bv": 0, "gamma": 32, "beta": 64}
    for nm, r in SROW.items():
        nc.sync.dma_start(out=smalls[r:r + 1, :], in_=ext[nm + "_ext"][0:1, :])
    smalls2 = const.tile([1, D], FP32, tag="smalls2", name="smalls2")
    nc.sync.dma_start(out=smalls2[0:1, :], in_=ext["beta_b2_ext"][0:1, :])

    res = [persist.tile([P, D], FP32, tag=f"res{m}", name=f"res{m}") for m in range(ST)]
    xT_f8 = persist.tile([P, DT, S_LOC], FP8, tag="xT", name="xT")
    qT_f8 = persist.tile([P, DT, S_LOC], FP8, tag="qT", name="qT")
    kT_f8 = persist.tile([P, DT, S_LOC], FP8, tag="kT", name="kT")
    v_half = [persist.tile([P, TJ, NF], FP8, tag=f"vf{h}", name=f"vf{h}")
              for h in range(2)]
    P_f8 = persist.tile([P, TJ, S_LOC], FP8, tag="pf", name="pf")

    # gather buffers are partition-major ([p, tile, col] flattened) so the
    # post-gather loads are single DMAs with 4KB-contiguous runs
    ag_q_in = [dram.tile([P, DT * NF], FP8, name=f"agqi{h}") for h in range(2)]
    ag_q_out = [dram.tile([G * P, DT * NF], FP8, name=f"agqo{h}") for h in range(2)]
    ag_v_in = [dram.tile([P, ST * NF], FP8, name=f"agvi{h}") for h in range(2)]
    ag_v_out = [dram.tile([G * P, ST * NF], FP8, name=f"agvo{h}") for h in range(2)]

    bcast = {}
    recipT = const.tile([P, ST], FP32, tag="recipT", name="recipT")
    lnt = const.tile([P, 16], FP32, tag="lnt", name="lnt")

    def load_w8(ext_t, base_row):
        tiles = []
        for kp in range(KP):
            wt = stream.tile([P, 2, D], FP8, tag=f"w{kp}", name=f"w{kp}")
            r0 = base_row + kp * P
            nc.sync.dma_start(out=wt[:, :, :], in_=ext_t[r0:r0 + P, :])
            tiles.append(wt)
        return tiles

    def ln_stats(st):
        # fresh stats tile per call (a shared tile would WAR-serialize the
        # per-row LN chains against each other)
        lt = stage.tile([P, 16], FP32, tag="lnt", name="lnt", bufs=3)
        stats = lt[:, 0:12]
        nc.vector.bn_stats(stats[:, 0:6], res[st][:, 0:NF])
        nc.vector.bn_stats(stats[:, 6:12], res[st][:, NF:2 * NF])
        mv = lt[:, 12:14]
        nc.vector.bn_aggr(mv[:], stats[:])
        negmu = lt[:, 14:15]
        nc.vector.tensor_scalar_mul(negmu[:], mv[:, 0:1], -1.0)
        sd = lt[:, 15:16]
        nc.scalar.activation(sd[:], mv[:, 1:2], AF.Sqrt, bias=eps_t[:])
        nc.vector.reciprocal(sd[:], sd[:])
        return negmu, sd

    def transpose_to(mmp, src_bf, dst_f8, s0):
        tp = mmp.tile([P, DT * P], BF16, tag="trp", name="trp", bufs=1)
        for dj in range(DT):
            nc.tensor.transpose(
                tp[:, dj * P:(dj + 1) * P], src_bf[:, dj * P:(dj + 1) * P],
                ident_bf[:],
            )
        nc.vector.tensor_copy(
            out=dst_f8[:, :, s0:s0 + P],
            in_=tp[:].rearrange("p (d s) -> p d s", d=DT),
        )

    # ================= phase A: QKV, attention, LN1, FFN1 =================
    with tcx.tile_pool(name="psA", bufs=1, space="PSUM") as mmp:
        # ---- x -> xT fp8: first s-half, then q-half0 can go ----
        def load_x_half(h):
            for si in range(h * 4, h * 4 + 4):
                xn = stage.tile([P, D], FP32, tag="stgf", name="stgf")
                nc.sync.dma_start(out=xn[:], in_=ext["x_ext"][si * P:(si + 1) * P, :])
                xb = stage.tile([P, D], BF16, tag="stgb", name="stgb")
                nc.vector.tensor_copy(out=xb[:], in_=xn[:])
                transpose_to(mmp, xb, xT_f8, si * P)

        def q_half(h):
            n0 = h * NF
            for m in range(DT):
                pt = mmp.tile([P, NF], FP32, tag="mm", name="mm", bufs=4)
                for kp in range(KP):
                    nc.tensor.matmul(
                        pt[:], wq[kp][:, :, m * P:(m + 1) * P],
                        xT_f8[:, 2 * kp:2 * kp + 2, n0:n0 + NF],
                        start=(kp == 0), stop=(kp == KP - 1), perf_mode=DR,
                    )
                nc.scalar.activation(qT_f8[:, m, n0:n0 + NF], pt[:], AF.Identity,
                                     bias=bq_sb[m])
                nc.sync.dma_start(
                    out=ag_q_in[h][:, m * NF:(m + 1) * NF],
                    in_=qT_f8[:, m, n0:n0 + NF],
                )
            nc.gpsimd.collective_compute(
                "AllGather", AluOpType.bypass, replica_groups=GROUPS,
                ins=[ag_q_in[h][:].opt()], outs=[ag_q_out[h][:].opt()],
            )

        load_x_half(0)
        wq = load_w8(ext["wq8_ext"], 0)
        q_half(0)
        load_x_half(1)
        q_half(1)

        # ---- v = x @ (32 Wv) + 32 bv (natural, fp8); AllGather (CC slot 3) ----
        wv = load_w8(ext["wv8_ext"], 0)
        bv_b = const.tile([P, D], FP32, tag="bc_bv", name="bc_bv")
        for n0 in range(0, D, NF):
            pt = mmp.tile([P, NF], FP32, tag="mm", name="mm", bufs=4)
            nc.tensor.matmul(pt[:], ones_f32[0:1, :], smalls[0:1, n0:n0 + NF])
            nc.scalar.copy(out=bv_b[:, n0:n0 + NF], in_=pt[:])
        for mt in range(ST):
            v8 = stage.tile([P, D], FP8, tag="v8", name="v8")
            for n0 in range(0, D, NF):
                pt = mmp.tile([P, NF], FP32, tag="mm", name="mm", bufs=4)
                for kp in range(KP):
                    nc.tensor.matmul(
                        pt[:], xT_f8[:, 2 * kp:2 * kp + 2, mt * P:(mt + 1) * P],
                        wv[kp][:, :, n0:n0 + NF],
                        start=(kp == 0), stop=(kp == KP - 1), perf_mode=DR,
                    )
                nc.vector.tensor_add(
                    v8[:, n0:n0 + NF], pt[:], bv_b[:, n0:n0 + NF]
                )
            for hh in range(2):
                nc.sync.dma_start(
                    out=ag_v_in[hh][:, mt * NF:(mt + 1) * NF],
                    in_=v8[:, hh * NF:(hh + 1) * NF],
                )
        # v gathered in two d-halves so pass B's first half can start while
        # the second half is still on the wire (the CC stream is serial)
        for hh in range(2):
            nc.gpsimd.collective_compute(
                "AllGather", AluOpType.bypass, replica_groups=GROUPS,
                ins=[ag_v_in[hh][:].opt()], outs=[ag_v_out[hh][:].opt()],
            )

        # ---- kT = (32 Wk).T @ x + 32 bk (fp8, local) ----
        wk = load_w8(ext["wk8_ext"], 0)
        for m in range(DT):
            for n0 in range(0, S_LOC, NF):
                pt = mmp.tile([P, NF], FP32, tag="mm", name="mm", bufs=4)
                for kp in range(KP):
                    nc.tensor.matmul(
                        pt[:], wk[kp][:, :, m * P:(m + 1) * P],
                        xT_f8[:, 2 * kp:2 * kp + 2, n0:n0 + NF],
                        start=(kp == 0), stop=(kp == KP - 1), perf_mode=DR,
                    )
                nc.scalar.activation(kT_f8[:, m, n0:n0 + NF], pt[:], AF.Identity,
                                     bias=bk_sb[m])

        # [P, D] broadcasts, off the critical path (fills AG wait)
        bc_rows = [("gamma", smalls[32:33, :], ones_f32[0:1, :]),
                   ("beta", smalls[64:65, :], ones_f32[0:1, :]),
                   ("beta_b2", smalls2[0:1, :], ones_f32[0:1, :])]
        if trivial_gb:
            bc_rows = [bc_rows[2]]  # only beta+b2 needed
        for nm, srow, orow in bc_rows:
            bt = const.tile([P, D], FP32, tag=f"bc_{nm}", name=f"bc_{nm}")
            for n0 in range(0, D, NF):
                pt = mmp.tile([P, NF], FP32, tag="mm", name="mm", bufs=4)
                if nm in ("gamma", "beta"):
                    nc.tensor.matmul(pt[:], ones_f32[0:1, :],
                                     smalls[SROW[nm]:SROW[nm] + 1, n0:n0 + NF])
                else:
                    nc.tensor.matmul(pt[:], orow, srow[:, n0:n0 + NF])
                nc.scalar.copy(out=bt[:, n0:n0 + NF], in_=pt[:])
            bcast[nm] = bt

        # ---- pass A: P[t, s] = exp(k·q/sqrt(D)); DR rowsums 1 chunk back ----
        rs_ps = [mmp.tile([1, NF], FP32, tag=f"rs{h}", name=f"rs{h}", bufs=1)
                 for h in range(2)]
        chunks = [(ht, r) for ht in range(2) for r in range(G)]

        def emit_rowsum(ci):
            ht, r = chunks[ci]
            jp0 = (r * ST + ht * 4) // 2
            for h in range(2):
                n0 = h * NF
                for jj in range(2):
                    a = 2 * ci + jj
                    nc.tensor.matmul(
                        rs_ps[h][:], ones_dr[:, :, 0:1],
                        P_f8[:, 2 * (jp0 + jj):2 * (jp0 + jj) + 2, n0:n0 + NF],
                        start=(a == 0), stop=(a == 2 * len(chunks) - 1),
                        perf_mode=DR,
                    )

        qtiles = {}

        def issue_qch(ci):
            ht, r = chunks[ci]
            qch = stream.tile([P, DT, NF], FP8, tag="q", name="q", bufs=3)
            nc.sync.dma_start(
                out=qch[:, :, :], in_=ag_q_out[ht][r * P:(r + 1) * P, :]
            )
            qtiles[ci] = qch

        issue_qch(0)
        for ci, (ht, r) in enumerate(chunks):
            if ci + 1 < len(chunks):
                issue_qch(ci + 1)
            qch = qtiles.pop(ci)
            for tti in range(4):
                j = r * ST + ht * 4 + tti
                for n0 in range(0, S_LOC, NF):
                    ps = mmp.tile([P, NF], FP32, tag="mm", name="mm", bufs=4)
                    for kp in range(KP):
                        nc.tensor.matmul(
                            ps[:], qch[:, 2 * kp:2 * kp + 2, tti * P:(tti + 1) * P],
                            kT_f8[:, 2 * kp:2 * kp + 2, n0:n0 + NF],
                            start=(kp == 0), stop=(kp == KP - 1), perf_mode=DR,
                        )
                    nc.scalar.activation(
                        P_f8[:, j, n0:n0 + NF], ps[:], AF.Exp, scale=EXP_SCALE
                    )
            if ci > 0:
                emit_rowsum(ci - 1)
        emit_rowsum(len(chunks) - 1)

        # recip of rowsums -> rs_row; the tiny transpose to per-partition form
        # is emitted inside pass B (after st0's matmuls) so the PE queue
        # doesn't stall on it before the attention matmuls can start.
        rs_row = const.tile([1, S_LOC], FP32, tag="rs_row", name="rs_row")
        for h in range(2):
            nc.vector.reciprocal(rs_row[0:1, h * NF:(h + 1) * NF], rs_ps[h][:])
        rs8 = const.tile([ST, P], FP32, tag="rs8", name="rs8")
        nc.scalar.dma_start(out=rs8[:, :], in_=rs_row[0:1, :])

        # ---- pass B: attn natural [s, d] + residual -> res (fp32) ----
        # d-half outer: half 0 computes while v's half-1 gather is in flight.
        # The two v loads go to different DMA queues so neither blocks the
        # other or the xre loads behind them.
        # both on the sync queue: on scalar their AG-wait would block the
        # xre loads (and pass-A exps) queued behind them
        for hh in range(2):
            nc.sync.dma_start(
                out=v_half[hh][:, :, :].rearrange("p (r m) c -> p r (m c)", r=G),
                in_=ag_v_out[hh][:, :].rearrange("(r p) c -> p r c", p=P),
            )
        for h in range(2):
            n0 = h * NF
            for st in range(ST):
                xre = stage.tile([P, NF], FP32, tag="xre", name="xre")
                nc.scalar.dma_start(
                    out=xre[:], in_=ext["x_ext"][st * P:(st + 1) * P, n0:n0 + NF]
                )
                ps = mmp.tile([P, NF], FP32, tag="mm", name="mm", bufs=4)
                for jp in range(TJ // 2):
                    nc.tensor.matmul(
                        ps[:], P_f8[:, 2 * jp:2 * jp + 2, st * P:(st + 1) * P],
                        v_half[h][:, 2 * jp:2 * jp + 2, :],
                        start=(jp == 0), stop=(jp == TJ // 2 - 1), perf_mode=DR,
                    )
                if h == 0 and st == 0:
                    rt_ps = mmp.tile([P, NF], FP32, tag="mm", name="mm", bufs=4)
                    nc.tensor.transpose(rt_ps[:, 0:ST], rs8[:, :],
                                        ident_f[0:ST, 0:ST])
                    nc.scalar.activation(recipT[:], rt_ps[:, 0:ST], AF.Identity,
                                         scale=1.0 / SCL)
                nc.vector.scalar_tensor_tensor(
                    out=res[st][:, n0:n0 + NF], in0=ps[:], scalar=recipT[:, st:st + 1],
                    in1=xre[:], op0=AluOpType.mult, op1=AluOpType.add,
                )

        # ---- LN1 (stats only -> res = z); x1T fp8; FFN1 per s-half ----
        # h stored per s-half, aliasing the two dead v half-tiles
        x1T_f8 = persist.tile([P, DT, S_LOC], FP8, tag="xT", name="xT")
        h_sh = [persist.tile([P, TJ, NF], FP8, tag=f"vf{h}", name=f"vf{h}")
                for h in range(2)]

        def ln1(st):
            negmu, sd = ln_stats(st)
            nc.vector.tensor_scalar(
                res[st][:], res[st][:], negmu[:], sd[:],
                op0=AluOpType.add, op1=AluOpType.mult,
            )
            xb = stage.tile([P, D], BF16, tag="stgb", name="stgb")
            nc.vector.tensor_copy(out=xb[:], in_=res[st][:])
            transpose_to(mmp, xb, x1T_f8, st * P)

        def ffn1_half(sh):
            n0 = sh * NF
            for g in range(HG):
                w1g = load_w8(ext["w18_ext"], g * KP * P)
                for mh_i in range(HPG):
                    mh = g * HPG + mh_i
                    pt = mmp.tile([P, NF], FP32, tag="mm", name="mm", bufs=4)
                    for kp in range(KP):
                        nc.tensor.matmul(
                            pt[:], w1g[kp][:, :, mh_i * P:(mh_i + 1) * P],
                            x1T_f8[:, 2 * kp:2 * kp + 2, n0:n0 + NF],
                            start=(kp == 0), stop=(kp == KP - 1), perf_mode=DR,
                        )
                    nc.scalar.activation(
                        h_sh[sh][:, mh, :], pt[:], AF.Gelu,
                        bias=b1_sb[mh], scale=1.0 / SCL,
                    )

        for st in range(4):
            ln1(st)
        ffn1_half(0)
        for st in range(4, ST):
            ln1(st)
        ffn1_half(1)

    # ================= phase B: FFN2 (fp8 DR) + LN2 + out =================
    # 4 passes of 2 s-tiles, alternating PSUM bank halves: pass p+1's matmuls
    # overlap pass p's vector epilogues, and only the last pass's tail shows.
    with tcx.tile_pool(name="psB", bufs=1, space="PSUM") as f2p:
        passes = [(0, 1), (2, 3), (4, 5), (6, 7)]
        for sp, sts in enumerate(passes):
            bk = 2 * (sp % 2)
            f2 = {(st, h): f2p.tile([P, NF], FP32, tag=f"f{i + bk}_{h}",
                                    name=f"f{i + bk}_{h}")
                  for i, st in enumerate(sts) for h in range(2)}

            def f2mm(kp2, st, h, w2t):
                nc.tensor.matmul(
                    f2[(st, h)][:],
                    h_sh[st // 4][:, 2 * kp2:2 * kp2 + 2,
                                  (st % 4) * P:(st % 4 + 1) * P],
                    w2t[:, :, h * NF:(h + 1) * NF],
                    start=(kp2 == 0), stop=(kp2 == HT // 2 - 1), perf_mode=DR,
                )

            w2_last = None
            for kp2 in range(HT // 2):
                wt = stream.tile([P, 2, D], FP8, tag=f"w{kp2 % KP}",
                                 name=f"w{kp2 % KP}")
                nc.sync.dma_start(
                    out=wt[:, :, :],
                    in_=ext["w28_ext"][kp2 * P:(kp2 + 1) * P, :],
                )
                if kp2 == 0 and trivial_gb:
                    # pre-add beta+b2 into res here (vector is idle during the
                    # matmul stream) so the per-tile tail epilogue shrinks
                    for st in sts:
                        nc.vector.tensor_add(res[st][:], res[st][:],
                                             bcast["beta_b2"][:])
                if kp2 < HT // 2 - 1:
                    for st in sts:
                        for h in range(2):
                            f2mm(kp2, st, h, wt)
                else:
                    w2_last = wt
            # last k-pair: finish one s-tile at a time and stream its epilogue
            for st in sts:
                for h in range(2):
                    f2mm(HT // 2 - 1, st, h, w2_last)
                # pre-LN2 = x1 + ff + b2 = z*gamma + (beta+b2) + f2/SCL2
                if trivial_gb:
                    for h in range(2):
                        n0 = h * NF
                        nc.vector.scalar_tensor_tensor(
                            out=res[st][:, n0:n0 + NF], in0=f2[(st, h)][:],
                            scalar=1.0 / SCL2, in1=res[st][:, n0:n0 + NF],
                            op0=AluOpType.mult, op1=AluOpType.add,
                        )
                else:
                    t2 = stage.tile([P, D], FP32, tag="stgf2", name="stgf2")
                    nc.vector.tensor_mul(t2[:], res[st][:], bcast["gamma"][:])
                    for h in range(2):
                        n0 = h * NF
                        nc.vector.scalar_tensor_tensor(
                            out=t2[:, n0:n0 + NF], in0=f2[(st, h)][:],
                            scalar=1.0 / SCL2, in1=t2[:, n0:n0 + NF],
                            op0=AluOpType.mult, op1=AluOpType.add,
                        )
                    nc.vector.tensor_add(res[st][:], t2[:], bcast["beta_b2"][:])
                # LN2 + store
                negmu, sd = ln_stats(st)
                ot = stage.tile([P, D], FP32, tag="stgf", name="stgf")
                nc.vector.tensor_scalar(
                    ot[:], res[st][:], negmu[:], sd[:],
                    op0=AluOpType.add, op1=AluOpType.mult,
                )
                if not trivial_gb:
                    nc.vector.tensor_mul(ot[:], ot[:], bcast["gamma"][:])
                    nc.vector.tensor_add(ot[:], ot[:], bcast["beta"][:])
                nc.sync.dma_start(
                    out=ext["out_ext"][st * P:(st + 1) * P, :], in_=ot[:]
                )


def build_nc(trivial_gb):
    nc = bacc.Bacc(target_bir_lowering=False, num_devices=N_CORES)

    ext = {
        "x_ext": nc.declare_dram_parameter("x", [S_LOC, D], FP32, isOutput=False),
        "wq8_ext": nc.declare_dram_parameter("wq8", [KP * P, 2 * D], FP8, isOutput=False),
        "wk8_ext": nc.declare_dram_parameter("wk8", [KP * P, 2 * D], FP8, isOutput=False),
        "wv8_ext": nc.declare_dram_parameter("wv8", [KP * P, 2 * D], FP8, isOutput=False),
        "w18_ext": nc.declare_dram_parameter("w18", [HG * KP * P, 2 * D], FP8, isOutput=False),
        "w28_ext": nc.declare_dram_parameter("w28", [(HT // 2) * P, 2 * D], FP8, isOutput=False),
        "bqp_ext": nc.declare_dram_parameter("bqp", [P, DT], FP32, isOutput=False),
        "bkp_ext": nc.declare_dram_parameter("bkp", [P, DT], FP32, isOutput=False),
        "b1p_ext": nc.declare_dram_parameter("b1p", [P, HT], FP32, isOutput=False),
        "bv_ext": nc.declare_dram_parameter("bv", [1, D], FP32, isOutput=False),
        "beta_b2_ext": nc.declare_dram_parameter("beta_b2", [1, D], FP32, isOutput=False),
        "gamma_ext": nc.declare_dram_parameter("gamma", [1, D], FP32, isOutput=False),
        "beta_ext": nc.declare_dram_parameter("beta", [1, D], FP32, isOutput=False),
        "out_ext": nc.declare_dram_parameter("out", [S_LOC, D], FP32, isOutput=True),
    }

    with tile.TileContext(nc) as tc:
        with (
            tc.tile_pool(name="dram", bufs=1, space="DRAM") as dram,
            tc.tile_pool(name="const", bufs=1) as const,
            tc.tile_pool(name="persist", bufs=1) as persist,
            tc.tile_pool(name="stage", bufs=2) as stage,
            tc.tile_pool(name="stream", bufs=2) as stream,
        ):
            ext.update(tc=tc, dram=dram, const=const, persist=persist,
                       stage=stage, stream=stream)
            build_graph(nc, tc, ext, trivial_gb)
    nc.compile()
    return nc


_NC_CACHE = {}


def _get_nc(trivial_gb):
    if trivial_gb not in _NC_CACHE:
        _NC_CACHE[trivial_gb] = build_nc(trivial_gb)
    return _NC_CACHE[trivial_gb]


F8NP = ml_dtypes.float8_e4m3


def _pair_rows(w):
    # [K, N] -> pair layout: rows kp*128+p, cols i*N+c = w[(2kp+i)*128+p, c]
    k, n = w.shape
    kp = k // (2 * P)
    w4 = w.reshape(kp, 2, P, n).transpose(0, 2, 1, 3).reshape(kp * P, 2 * n)
    return np.ascontiguousarray(w4)


def _col_pack(v, n):
    # [n*128] -> [128, n] with out[p, m] = v[m*128 + p]
    return np.ascontiguousarray(v.reshape(n, P).T)


def _make_in_maps(inputs):
    x = np.asarray(inputs["input_embedding"], dtype=np.float32)
    assert x.shape == (B, S, D), x.shape

    gamma = np.asarray(inputs["gamma"], np.float32).reshape(D)
    beta = np.asarray(inputs["beta"], np.float32).reshape(D)
    trivial_gb = bool(np.all(gamma == 1.0) and np.all(beta == 0.0))
    W1 = np.asarray(inputs["W1"], np.float32)
    b1 = np.asarray(inputs["b1"], np.float32).reshape(H)
    # fold LN1's gamma/beta into W1/b1 (FFN1 consumes the normalized z)
    W1f = gamma[:, None] * W1
    b1f = b1 + beta @ W1
    # W1 group-major pair layout: rows (g*KP+kp)*128+p, cols i*D+c
    w1g = (SCL * W1f).reshape(KP, 2, P, HG, D).transpose(3, 0, 2, 1, 4)
    w18 = np.ascontiguousarray(w1g.reshape(HG * KP * P, 2 * D)).astype(F8NP)

    shared = {
        "wq8": _pair_rows(SCL * np.asarray(inputs["Wq"], np.float32)).astype(F8NP),
        "wk8": _pair_rows(SCL * np.asarray(inputs["Wk"], np.float32)).astype(F8NP),
        "wv8": _pair_rows(SCL * np.asarray(inputs["Wv"], np.float32)).astype(F8NP),
        "w18": w18,
        "w28": _pair_rows(SCL2 * np.asarray(inputs["W2"], np.float32)).astype(F8NP),
        "bqp": _col_pack(SCL * np.asarray(inputs["bq"], np.float32).reshape(D), DT),
        "bkp": _col_pack(SCL * np.asarray(inputs["bk"], np.float32).reshape(D), DT),
        "b1p": _col_pack(b1f, HT),
        "bv": SCL * np.asarray(inputs["bv"], np.float32).reshape(1, D),
        "beta_b2": (beta + np.asarray(inputs["b2"], np.float32).reshape(D)).reshape(1, D),
        "gamma": gamma.reshape(1, D),
        "beta": beta.reshape(1, D),
    }

    in_maps = []
    for c in range(N_CORES):
        b = c // G
        r = c % G
        m = dict(shared)
        m["x"] = np.ascontiguousarray(x[b, r * S_LOC:(r + 1) * S_LOC, :])
        in_maps.append(m)
    return in_maps, trivial_gb


def kernel(**inputs: np.ndarray) -> np.ndarray:
    from concourse.bass_utils import run_bass_kernel_spmd

    in_maps, trivial_gb = _make_in_maps(inputs)
    nc = _get_nc(trivial_gb)
    res = run_bass_kernel_spmd(nc, in_maps, core_ids=list(range(N_CORES)))

    out = np.empty((B, S, D), dtype=np.float32)
    for c in range(N_CORES):
        b = c // G
        r = c % G
        out[b, r * S_LOC:(r + 1) * S_LOC, :] = res.results[c]["out"]
    return out
